# revision 53
# baseline (speedup 1.0000x reference)
"""Trainium2 Bass kernel for nn_GPT_61409442398424 (4-layer spiking GPT).

Sharding: DP-2 over batch (core groups {0-3},{4-7}) x TP-4 within group
(Wq/Wk/Wv by heads, Wfc/Wp by hidden dim, uni by HASH rows for logits).

v3 design notes (on top of v2), measured 2435us -> 1623us:
- Token-halved pipeline: every layer phase (QKV/rope/LIF/attention/Wo/
  AllReduce/MLP) runs per 512-token half so each AllReduce overlaps the
  other half's compute. ARs stay 1MB bf16 (mesh regime, ~33us each).
- LIF fixpoint (KFIX=7, adds ~7e-3 rel err; K=6 would breach the 2e-2
  gate) runs per half with carry chaining via tensor_tensor_scan's AP
  `initial`. Scans/STT are DVE-only (GpSimd lacks both; its
  tensor_scalar ucode is ~7us per [128,512] op - avoid).
- Half-b QKV is precomputed against pre-AllReduce x during the fix_a
  window, then a delta pass folds in rm0*msc*AR2b after it lands, so the
  post-AR critical path only carries 24 delta matmuls + rope.
- Attention software-pipelined: exp'd score tiles produced LOOK=3 tiles
  ahead of the PV matmuls, heads interleaved; the causal tri mask runs
  on GpSimd so attention never queues behind the other half's LIF.
- Weights host-packed into exact tile layouts (>=1KB contiguous per
  partition per descriptor); wp/unit/h/logits in bf16; the q-rms,
  attention-denominator, and norm-row partition-broadcast matmuls are
  single bf16/f32 [2,SH]-row matmuls instead of pairs of 4cyc/row f32.
- AR-consume tiles use a separate pool from outbound evac tiles and
  their DMAs issue from the Scalar queue; outbound bf16 evacs DMA from
  the GpSimd queue (a dma_start costs ~0.6us of its queue).
- PE runs at K=4/8 (1.2GHz) for ~60% of the span due to the HAM activity
  throttle + board GPIO caps; dense 12us+ matmul runs still measure
  cold, so further scheduling densification has limited return.
"""
import os
import numpy as np

import concourse.bass as bass
import concourse.tile as tile
from concourse import bacc, mybir
from concourse.bass_utils import run_bass_kernel_spmd

F32 = mybir.dt.float32
F32R = mybir.dt.float32r
BF16 = mybir.dt.bfloat16
AB = mybir.AluOpType
AFT = mybir.ActivationFunctionType
BF16_NP = mybir.dt.np(mybir.dt.bfloat16)

B, S, DM, H, HKV, L, MLP_MULT = 2, 1024, 1024, 16, 4, 4, 4
DH = DM // H
HASH, VOCAB = 16384, 50257
EPS = 1.1920929e-07
THRESH, DECAY = 0.8, 0.9
ROPE_BASE = 10000.0
N_CORES = 8
TP = 4
HEADS_PC = H // TP        # 4 q heads per core
QD = HEADS_PC * DH        # 256 q dims per core
KD = DH                   # 64 kv dims per core (1 kv head)
HID_PC = MLP_MULT * DM // TP
HASH_PC = HASH // TP
NT = S // 128
ND = DM // 128
SH = S // 2               # tokens per half
KFIX = 7                  # LIF fixpoint scans per half

_CACHE = {}


def build_program():
    nc = bacc.Bacc("TRN2", target_bir_lowering=False, debug=False,
                   enable_asserts=False, num_devices=N_CORES)

    din = {}
    def di(name, shape, dt=F32R):
        din[name] = nc.dram_tensor(name, shape, dt, kind="ExternalInput").ap()
        return din[name]

    xe1 = di("xe1", [DM, S], F32R)
    xe2 = di("xe2", [DM, S], F32R)
    wqkv = di("wqkv", [L, 128, ND, QD + 2 * KD], F32R)  # tile-packed
    wo = di("wo", [L, ND, 128, 2, 128], F32R)           # [l,d,p,c,f]
    wfc = di("wfc", [L, ND, 128, ND, 128], F32R)        # [l,hh,p,d,f]
    wp = di("wp", [L, ND, 128, ND, 128], BF16)          # [l,d,p,hh,f]
    unit = di("unit", [DM, HASH_PC], BF16)
    cosq = di("cosq", [128, S], F32)     # q-tile rope tables (2 heads/tile)
    sinq = di("sinq", [128, S], F32)     # signed
    cosk = di("cosk", [64, S], F32)
    sink = di("sink", [64, S], F32)
    pswp = di("pswp", [128, 2, 128], F32R)  # [:,0,:]=Pq ; [0:64,1,0:64]=Pk
    ident = di("ident", [128, 128], F32R)
    tri = di("tri", [128, 128], F32R)
    selT = di("selT", [128, 2], F32R)
    sel2f = di("sel2f", [2, 128], F32)
    scal = di("scal", [128, ND, 5 * L], F32)
    qgain = di("qgain", [128, 2, L], F32)
    out_lg = nc.dram_tensor("out_lg", [S, HASH_PC], BF16, kind="ExternalOutput").ap()

    # ---------------- persistent SBUF ------------------------------------
    x_t = [nc.alloc_sbuf_tensor(f"x_{d}", [128, S], F32R) for d in range(ND)]
    xn_t = [nc.alloc_sbuf_tensor(f"xn_{d}", [128, SH], F32R) for d in range(ND)]
    h_t = [nc.alloc_sbuf_tensor(f"h_{d}", [128, S], BF16) for d in range(ND)]
    qsb = [nc.alloc_sbuf_tensor(f"qsb_{j}", [128, S], F32R) for j in range(2)]
    kvsb = nc.alloc_sbuf_tensor("kvsb", [128, SH], F32R)
    q4 = [nc.alloc_sbuf_tensor(f"q4_{j}", [128, S], F32R) for j in range(2)]
    u2 = [nc.alloc_sbuf_tensor(f"u2_{j}", [128, S], F32) for j in range(2)]
    c2 = [nc.alloc_sbuf_tensor(f"c2_{j}", [128, S + 1], F32) for j in range(2)]
    e2 = [nc.alloc_sbuf_tensor(f"e2_{j}", [128, SH], F32) for j in range(2)]
    et2 = [nc.alloc_sbuf_tensor(f"et2_{j}", [128, SH], F32) for j in range(2)]
    yt2 = [nc.alloc_sbuf_tensor(f"yt2_{j}", [128, SH], F32R) for j in range(2)]
    v65 = nc.alloc_sbuf_tensor("v65", [128, NT, 65], F32R)
    kt2 = nc.alloc_sbuf_tensor("kt2", [128, S], F32R)
    bc_sb = nc.alloc_sbuf_tensor("bc_sb", [128, SH], F32)
    wqkv_s = nc.alloc_sbuf_tensor("wqkv_s", [128, ND, QD + 2 * KD], F32R)
    cosq_s = nc.alloc_sbuf_tensor("cosq_s", [128, S], F32)
    sinq_s = nc.alloc_sbuf_tensor("sinq_s", [128, S], F32)
    cosk_s = nc.alloc_sbuf_tensor("cosk_s", [64, S], F32)
    sink_s = nc.alloc_sbuf_tensor("sink_s", [64, S], F32)
    pswp_s = nc.alloc_sbuf_tensor("pswp_s", [128, 2, 128], F32R)
    ident_s = nc.alloc_sbuf_tensor("ident_s", [128, 128], F32R)
    tri_s = nc.alloc_sbuf_tensor("tri_s", [128, 128], F32R)
    scal_s = nc.alloc_sbuf_tensor("scal_s", [128, ND, 5 * L], F32)
    qgain_s = nc.alloc_sbuf_tensor("qgain_s", [128, 2, L], F32)
    rkc = nc.alloc_sbuf_tensor("rkc", [128, NT], F32)    # 0.125/rms(k) per key
    lnbc = nc.alloc_sbuf_tensor("lnbc", [128, NT], F32)  # ln(bc) per key
    ibc = nc.alloc_sbuf_tensor("ibc", [128, NT], F32R)   # 1/bc per key
    rows_sb = nc.alloc_sbuf_tensor("rows_sb", [128, S], F32)
    hrows = nc.alloc_sbuf_tensor("hrows", [2, SH], F32)
    drows = nc.alloc_sbuf_tensor("drows", [33, SH], BF16)
    selT_s = nc.alloc_sbuf_tensor("selT_s", [128, 2], F32R)
    sel2f_s = nc.alloc_sbuf_tensor("sel2f_s", [2, 128], F32)
    onesb = nc.alloc_sbuf_tensor("onesb", [33, 128], BF16)
    brow = nc.alloc_sbuf_tensor("brow", [1, SH], BF16)
    onesr_f = nc.alloc_sbuf_tensor("onesr_f", [128, 128], F32)
    onesr = nc.alloc_sbuf_tensor("onesr", [128, 128], F32R)
    onesc_f = nc.alloc_sbuf_tensor("onesc_f", [128, 1], F32)
    onesc = nc.alloc_sbuf_tensor("onesc", [128, 1], F32R)
    d9_s = nc.alloc_sbuf_tensor("d9_s", [128, 1], F32)
    mtmp = nc.alloc_sbuf_tensor("mtmp", [128, 1], F32)
    zc = nc.alloc_sbuf_tensor("zc", [128, 1], F32)
    epsc = nc.alloc_sbuf_tensor("epsc", [128, 1], F32)
    rl_row = rows_sb[0:1, :]         # 1/rms per token (norms)
    ln_row = rows_sb[64:65, :]       # ln per token (v bias)

    RG = [[0, 1, 2, 3], [4, 5, 6, 7]]

    with tile.TileContext(nc) as tc:
        with tc.tile_pool(name="gp", bufs=4) as gp, \
             tc.tile_pool(name="gpb", bufs=3) as gpb, \
             tc.tile_pool(name="etp", bufs=4) as etp, \
             tc.tile_pool(name="wop", bufs=3) as wop, \
             tc.tile_pool(name="wfp", bufs=3) as wfp, \
             tc.tile_pool(name="wpp", bufs=3) as wpp, \
             tc.tile_pool(name="unp", bufs=8) as unp, \
             tc.tile_pool(name="psB", bufs=6, space="PSUM") as psB, \
             tc.tile_pool(name="psY", bufs=2, space="PSUM") as psY, \
             tc.tile_pool(name="dram", bufs=1, space="DRAM") as dram:

            arb_i = dram.tile([DM, S], F32R)
            arb_p = [[dram.tile([DM, SH], BF16, name=f"arbp{c}{h}")
                      for h in range(2)] for c in range(2)]
            arb_o = [[dram.tile([DM, SH], BF16, name=f"arbo{c}{h}")
                      for h in range(2)] for c in range(2)]

            # ---- constants / tables ----
            nc.sync.dma_start(cosq_s[:], cosq[:])
            nc.sync.dma_start(sinq_s[:], sinq[:])
            nc.sync.dma_start(cosk_s[:], cosk[:])
            nc.sync.dma_start(sink_s[:], sink[:])
            nc.sync.dma_start(pswp_s[:], pswp[:])
            nc.sync.dma_start(ident_s[:], ident[:])
            nc.sync.dma_start(tri_s[:], tri[:])
            nc.sync.dma_start(selT_s[:], selT[:])
            nc.sync.dma_start(sel2f_s[:], sel2f[:])
            nc.sync.dma_start(scal_s[:], scal[:])
            nc.sync.dma_start(qgain_s[:], qgain[:])
            nc.sync.dma_start(wqkv_s[:], wqkv[0])
            nc.vector.memset(mtmp[:], 1.0)
            nc.vector.tensor_copy(onesc[:], mtmp[:])
            nc.vector.tensor_copy(onesr[:], mtmp[:, 0:1].to_broadcast((128, 128)))
            nc.vector.tensor_copy(onesr_f[:], mtmp[:, 0:1].to_broadcast((128, 128)))
            nc.vector.tensor_copy(onesb[:], mtmp[0:33, 0:1].to_broadcast((33, 128)))
            nc.vector.tensor_copy(onesc_f[:], mtmp[:])
            nc.vector.memset(d9_s[:], 0.9)
            nc.vector.memset(zc[:], 0.0)
            nc.vector.memset(epsc[:], EPS)

            # ---- embedding: x = xe1 + xe2 (also x0, kept in DRAM) ----
            for d in range(ND):
                ds = slice(128 * d, 128 * d + 128)
                nc.sync.dma_start(x_t[d][:], xe2[ds, :])
                for eh in range(2):
                    ecs = slice(SH * eh, SH * eh + SH)
                    t1 = gp.tile([128, SH], F32R, tag="gp")
                    nc.sync.dma_start(t1[:], xe1[ds, ecs])
                    nc.gpsimd.tensor_tensor(x_t[d][:, ecs], x_t[d][:, ecs],
                                            t1[:], AB.add)
                    nc.sync.dma_start(arb_i[ds, ecs], x_t[d][:, ecs])

            def cs_of(h):
                return slice(SH * h, SH * h + SH)

            def ssq_half(ps, h, src, split=False):
                """ps[0:1, 0:SH] = sum over DM of src^2 for token half h."""
                for d in range(ND):
                    sq = gp.tile([128, SH], F32R, tag="gp")
                    if split and d % 2 == 1:
                        nc.vector.tensor_tensor(sq[:], src[d][:, cs_of(h)],
                                                src[d][:, cs_of(h)], AB.mult)
                    else:
                        nc.scalar.activation(sq[:], src[d][:, cs_of(h)],
                                             AFT.Square, bias=zc[:])
                    nc.tensor.matmul(ps[0:1, 0:SH], onesc[:], sq[:],
                                     start=(d == 0), stop=(d == ND - 1))

            def lnbc_half(l, h):
                """Per-token ln(rsqrt(mean x^2+eps)) for v (exp bias), half h."""
                cs = cs_of(h)
                ssq_ps = psB.tile([128, SH], F32, tag="psB")
                ssq_half(ssq_ps, h, x_t)
                nc.scalar.activation(ln_row[:, cs], ssq_ps[0:1, 0:SH], AFT.Ln,
                                     bias=epsc[0:1, :], scale=1.0 / DM)
                nc.vector.tensor_scalar(ln_row[:, cs], ln_row[:, cs], -0.5,
                                        None, AB.mult)
                lnp = psB.tile([128, SH], F32, tag="psB")
                for tl in range(4):
                    t = 4 * h + tl
                    nc.tensor.transpose(lnp[:, tl:tl + 1],
                                        rows_sb[64:65, 128 * t:128 * t + 128]
                                        .bitcast(F32),
                                        ident_s[64:65, 64:65].bitcast(F32))
                nc.scalar.copy(lnbc[:, 4 * h:4 * h + 4], lnp[:, 0:4])
                nc.scalar.activation(ibc[:, 4 * h:4 * h + 4],
                                     lnbc[:, 4 * h:4 * h + 4], AFT.Exp,
                                     bias=zc[:], scale=-1.0)

            def rope_qk(l, h):
                """q-head rms + rope, k rope + rms, v transpose for half h."""
                cs = cs_of(h)
                for jt in range(2):
                    sq = gp.tile([128, SH], F32R, tag="gp")
                    nc.scalar.activation(sq[:], qsb[jt][:, cs], AFT.Square,
                                         bias=zc[:])
                    rq_ps = psB.tile([128, SH], F32, tag="psB")
                    nc.tensor.matmul(rq_ps[0:2, 0:SH], selT_s[:], sq[:],
                                     start=True, stop=True)
                    nc.scalar.activation(hrows[0:2, 0:SH], rq_ps[0:2, 0:SH],
                                         AFT.Sqrt, bias=zc[0:2, :],
                                         scale=1.0 / DH)
                    rqb = psB.tile([128, SH], F32, tag="psB")
                    nc.tensor.matmul(rqb[:, 0:SH], sel2f_s[:],
                                     hrows[0:2, 0:SH], start=True, stop=True)
                    rqi = gp.tile([128, SH], F32, tag="gp", name="rqi")
                    nc.vector.reciprocal_approx_fast(out=rqi[:],
                                                     in_=rqb[:, 0:SH])
                    swp = psB.tile([128, SH], F32, tag="psB")
                    nc.tensor.matmul(swp[:, 0:SH], pswp_s[:, 0, :],
                                     qsb[jt][:, cs], start=True, stop=True)
                    t1 = et2[0][:, 0:SH]
                    nc.vector.scalar_tensor_tensor(
                        t1, qsb[jt][:, cs], 1.0, cosq_s[:, cs],
                        AB.mult, AB.mult)
                    t2 = et2[1][:, 0:SH]
                    nc.vector.scalar_tensor_tensor(
                        t2, swp[:, 0:SH], 1.0, sinq_s[:, cs],
                        AB.mult, AB.mult)
                    nc.vector.scalar_tensor_tensor(
                        t1, t1, 1.0, t2, AB.mult, AB.add)
                    nc.vector.scalar_tensor_tensor(
                        qsb[jt][:, cs], t1, 1.0, rqi[:], AB.mult, AB.mult)

                # k rope
                swp = psB.tile([128, SH], F32, tag="psB")
                nc.tensor.matmul(swp[0:64, 0:SH], pswp_s[0:64, 1, 0:64],
                                 kvsb[0:64, 0:SH], start=True, stop=True)
                t1 = et2[0][0:64, 0:SH]
                nc.vector.scalar_tensor_tensor(
                    t1, kvsb[0:64, 0:SH], 1.0, cosk_s[:, cs],
                    AB.mult, AB.mult)
                t2 = et2[1][0:64, 0:SH]
                nc.vector.scalar_tensor_tensor(
                    t2, swp[0:64, 0:SH], 1.0, sink_s[:, cs],
                    AB.mult, AB.mult)
                nc.vector.scalar_tensor_tensor(
                    kt2[0:64, cs], t1, 1.0, t2, AB.mult, AB.add)
                nc.scalar.copy(kt2[64:128, cs], kt2[0:64, cs])

                # k-head rms -> per-key scale column (0.125/rms)
                ksq = gp.tile([128, SH], F32, tag="gp")
                nc.scalar.activation(ksq[0:64, :], kt2[0:64, cs], AFT.Square,
                                     bias=zc[0:64, :])
                rkp = psB.tile([128, SH], F32, tag="psB")
                for tl in range(4):
                    nc.tensor.matmul(rkp[:, tl:tl + 1],
                                     ksq[0:64, 128 * tl:128 * tl + 128],
                                     onesc_f[0:64, :],
                                     start=True, stop=True)
                tsl = slice(4 * h, 4 * h + 4)
                nc.scalar.activation(rkc[:, tsl], rkp[:, 0:4], AFT.Sqrt,
                                     bias=zc[:], scale=1.0 / DH)
                nc.vector.reciprocal_approx_fast(out=rkc[:, tsl],
                                                 in_=rkc[:, tsl])
                nc.vector.tensor_scalar(rkc[:, tsl], rkc[:, tsl], 0.125,
                                        None, AB.mult)
                # v -> token-major tiles via PE transpose
                for tl in range(4):
                    t = 4 * h + tl
                    vtp = psB.tile([128, SH], F32, tag="psB")
                    nc.tensor.transpose(vtp[:, 0:64].bitcast(F32R),
                                        kvsb[64:128, 128 * tl:128 * tl + 128],
                                        ident_s[64:128, 0:64])
                    nc.scalar.copy(v65[:, t, 0:64], vtp[:, 0:64])
                    nc.gpsimd.tensor_copy(v65[:, t, 64:65], ibc[:, t:t + 1])

            def resid_a(l):
                """AR2a consume + resid mix + QKV + lnbc + rope for half a."""
                cs = cs_of(0)
                for d in range(ND):
                    ds = slice(128 * d, 128 * d + 128)
                    rm0 = scal_s[:, d, 5 * l + 0:5 * l + 1]
                    rm1 = scal_s[:, d, 5 * l + 1:5 * l + 2]
                    if l == 0:
                        tt = gp.tile([128, SH], F32, tag="gp")
                        nc.scalar.mul(tt[:], x_t[d][:, cs], rm1)
                        nc.vector.scalar_tensor_tensor(
                            x_t[d][:, cs], x_t[d][:, cs], rm0, tt[:],
                            AB.mult, AB.add)
                    else:
                        x0t = gp.tile([128, SH], F32R, tag="gp")
                        nc.sync.dma_start(x0t[:], arb_i[ds, cs])
                        tt = gp.tile([128, SH], F32, tag="gp")
                        nc.scalar.mul(tt[:], x0t[:], rm1)
                        nc.vector.scalar_tensor_tensor(
                            x_t[d][:, cs], x_t[d][:, cs], rm0, tt[:],
                            AB.mult, AB.add)
                        art = wop.tile([128, SH], BF16, tag="wop")
                        nc.scalar.dma_start(art[:], arb_o[1][0][ds, :])
                        fold = scal_s[:, d, 5 * l + 4:5 * l + 5]
                        nc.vector.scalar_tensor_tensor(
                            x_t[d][:, cs], art[:], fold, x_t[d][:, cs],
                            AB.mult, AB.add)
                pss = [psB.tile([128, SH], F32, tag="psB", name=f"qkva{i}")
                       for i in range(3)]
                for d in range(ND):
                    for jt in range(3):
                        nc.tensor.matmul(
                            pss[jt][:, 0:SH],
                            wqkv_s[:, d, 128 * jt:128 * jt + 128],
                            x_t[d][:, cs],
                            start=(d == 0), stop=(d == ND - 1))
                for jt in range(2):
                    nc.scalar.copy(qsb[jt][:, cs], pss[jt][:, 0:SH])
                nc.scalar.copy(kvsb[:, 0:SH], pss[2][:, 0:SH])
                lnbc_half(l, 0)
                rope_qk(l, 0)

            def residmix_b(l):
                """x_b = rm0*x_b + rm1*x0_b (AR2b part folded in later)."""
                cs = cs_of(1)
                for d in range(ND):
                    ds = slice(128 * d, 128 * d + 128)
                    rm0 = scal_s[:, d, 5 * l + 0:5 * l + 1]
                    rm1 = scal_s[:, d, 5 * l + 1:5 * l + 2]
                    if l == 0:
                        tt = gp.tile([128, SH], F32, tag="gp")
                        nc.scalar.mul(tt[:], x_t[d][:, cs], rm1)
                        nc.vector.scalar_tensor_tensor(
                            x_t[d][:, cs], x_t[d][:, cs], rm0, tt[:],
                            AB.mult, AB.add)
                    else:
                        x0t = gp.tile([128, SH], F32R, tag="gp")
                        nc.sync.dma_start(x0t[:], arb_i[ds, cs])
                        tt = gp.tile([128, SH], F32, tag="gp")
                        nc.scalar.mul(tt[:], x0t[:], rm1)
                        nc.vector.scalar_tensor_tensor(
                            x_t[d][:, cs], x_t[d][:, cs], rm0, tt[:],
                            AB.mult, AB.add)

            def qkv_b_partial(l):
                """QKV over pre-AR x_b; for l=0 this is the whole thing."""
                cs = cs_of(1)
                pss = [psB.tile([128, SH], F32, tag="psB", name=f"qkvb{i}")
                       for i in range(3)]
                for d in range(ND):
                    for jt in range(3):
                        nc.tensor.matmul(
                            pss[jt][:, 0:SH],
                            wqkv_s[:, d, 128 * jt:128 * jt + 128],
                            x_t[d][:, cs],
                            start=(d == 0),
                            stop=(l == 0 and d == ND - 1))
                if l == 0:
                    for jt in range(2):
                        nc.scalar.copy(qsb[jt][:, cs], pss[jt][:, 0:SH])
                    nc.scalar.copy(kvsb[:, 0:SH], pss[2][:, 0:SH])
                return pss

            def consume_b(l, pss):
                """Fold AR2b into x_b and into the QKV_b psums (delta pass)."""
                if l == 0:
                    return
                cs = cs_of(1)
                for d in range(ND):
                    ds = slice(128 * d, 128 * d + 128)
                    art = wop.tile([128, SH], BF16, tag="wop")
                    nc.scalar.dma_start(art[:], arb_o[1][1][ds, :])
                    fold = scal_s[:, d, 5 * l + 4:5 * l + 5]
                    tmp = gp.tile([128, SH], F32R, tag="gp", name="artmp")
                    nc.scalar.mul(tmp[:], art[:], fold)
                    nc.vector.tensor_tensor(x_t[d][:, cs], x_t[d][:, cs],
                                            tmp[:], AB.add)
                    for jt in range(3):
                        nc.tensor.matmul(
                            pss[jt][:, 0:SH],
                            wqkv_s[:, d, 128 * jt:128 * jt + 128],
                            tmp[:],
                            start=False, stop=(d == ND - 1))
                for jt in range(2):
                    nc.scalar.copy(qsb[jt][:, cs], pss[jt][:, 0:SH])
                nc.scalar.copy(kvsb[:, 0:SH], pss[2][:, 0:SH])

            def lif_chain(l, h, j):
                """LIF fixpoint for half h, chain j (DVE)."""
                cs = cs_of(h)
                h0, h1 = SH * h, SH * h + SH
                d9 = d9_s[:].to_broadcast((128, SH))
                z9 = zc[:].to_broadcast((128, SH))
                if True:
                    eng = nc.vector
                    if h == 0:
                        eng.tensor_tensor_scan(u2[j][:, cs], d9, qsb[j][:, cs],
                                               0.0, AB.mult, AB.add)
                        eng.memset(c2[j][:, 0:1], 0.0)
                    else:
                        eng.tensor_tensor_scan(u2[j][:, cs], d9, qsb[j][:, cs],
                                               u2[j][:, h0 - 1:h0],
                                               AB.mult, AB.add)
                        # decay-only carry fill of c cols before iteration 0
                        eng.tensor_tensor_scan(c2[j][:, h0 + 1:h1 + 1], d9, z9,
                                               c2[j][:, h0:h0 + 1],
                                               AB.mult, AB.max)
                    for p in range(KFIX):
                        if p == 0 and h == 0:
                            eng.scalar_tensor_tensor(
                                e2[j][:], u2[j][:, cs], THRESH, u2[j][:, cs],
                                AB.is_ge, AB.mult)
                        else:
                            eng.scalar_tensor_tensor(
                                e2[j][:], c2[j][:, h0:h1], -DECAY,
                                u2[j][:, cs], AB.mult, AB.add)
                            eng.scalar_tensor_tensor(
                                e2[j][:], e2[j][:], THRESH, u2[j][:, cs],
                                AB.is_ge, AB.mult)
                        init = 0.0 if h == 0 else c2[j][:, h0:h0 + 1]
                        eng.tensor_tensor_scan(
                            c2[j][:, h0 + 1:h1 + 1], d9, e2[j][:], init,
                            AB.mult, AB.max)
                    # final spikes*gain; gated q -> q4
                    eng.scalar_tensor_tensor(
                        e2[j][:], c2[j][:, h0:h1], -DECAY, u2[j][:, cs],
                        AB.mult, AB.add)
                    eng.tensor_scalar(e2[j][:], e2[j][:], THRESH,
                                      qgain_s[:, j, l:l + 1],
                                      AB.is_ge, AB.mult)
                    eng.scalar_tensor_tensor(q4[j][:, cs], qsb[j][:, cs], 1.0,
                                             e2[j][:], AB.mult, AB.mult)

            def attn_half(l, h):
                """Attention for token half h (keys 0..(h+1)*SH), both chains.

                Software-pipelined: exp'd score tiles (et) are produced LOOK
                tiles ahead of the PV matmuls that consume them, with the two
                heads interleaved, so the PE never stalls on Exp latency.
                The causal tri mask runs on GpSimd (DVE is busy with the
                other half's LIF fixpoint)."""
                cs = cs_of(h)
                tlist = list(range(4 * (h + 1)))
                nmm = len(tlist)
                for j in range(2):
                    yups = [psY.tile([128, SH], F32, tag="psY", name=f"yup{i}")
                            for i in range(2)]
                    work = [(hl, t) for t in tlist for hl in range(2)]
                    ets = {}

                    def issue_score(idx):
                        hl, t = work[idx]
                        off = 64 * hl
                        tok0 = max(SH * h, 128 * t)
                        ncols = SH * h + SH - tok0
                        et = etp.tile([128, SH], F32R, tag="etp")
                        scp = psB.tile([128, SH], F32, tag="psB")
                        nc.tensor.matmul(
                            scp[:, 0:ncols],
                            kt2[off:off + 64, 128 * t:128 * t + 128],
                            q4[j][off:off + 64, tok0:SH * h + SH],
                            start=True, stop=True)
                        nc.scalar.activation(
                            et[:, 0:ncols], scp[:, 0:ncols],
                            AFT.Exp, bias=lnbc[:, t:t + 1],
                            scale=rkc[:, t:t + 1])
                        if 128 * t >= SH * h:
                            nc.gpsimd.tensor_tensor(
                                et[:, 0:128], et[:, 0:128], tri_s[:],
                                AB.mult)
                        ets[idx] = et

                    LOOK = 3
                    for idx in range(min(LOOK, len(work))):
                        issue_score(idx)
                    for idx, (hl, t) in enumerate(work):
                        if idx + LOOK < len(work):
                            issue_score(idx + LOOK)
                        et = ets.pop(idx)
                        tok0 = max(SH * h, 128 * t)
                        ncols = SH * h + SH - tok0
                        cols0 = tok0 - SH * h
                        i = tlist.index(t)
                        nc.tensor.matmul(yups[hl][0:65, cols0:SH],
                                         v65[:, t, :], et[:, 0:ncols],
                                         start=(i == 0), stop=(i == nmm - 1))
                    for hl in range(2):
                        off = 64 * hl
                        yup = yups[hl]
                        nc.scalar.copy(q4[j][off:off + 64, cs], yup[0:64, :])
                        nc.scalar.copy(drows[32 * hl:32 * hl + 1, 0:SH],
                                       yup[64:65, :])
                    # epilogue: divide by denominator broadcast
                    rbp = psB.tile([128, SH], F32, tag="psB")
                    for hl in range(2):
                        nc.tensor.matmul(rbp[64 * hl:64 * hl + 64, 0:SH],
                                         onesb[32 * hl:32 * hl + 1, 0:64],
                                         drows[32 * hl:32 * hl + 1, 0:SH],
                                         start=True, stop=True)
                    rbi = gp.tile([128, SH], F32, tag="gp", name="rbi")
                    nc.vector.reciprocal_approx_fast(out=rbi[:], in_=rbp[:, 0:SH])
                    nc.vector.scalar_tensor_tensor(yt2[j][:, 0:SH],
                                                   q4[j][:, cs], 1.0,
                                                   rbi[:], AB.mult, AB.mult)

            def wo_half(l, h):
                """Wo partials for half h -> bf16 bounce; caller triggers AR."""
                cs = cs_of(h)
                for d in range(ND):
                    aop = psB.tile([128, SH], F32, tag="psB")
                    wt = wop.tile([128, 2, 128], F32R, tag="wop")
                    nc.sync.dma_start(wt[:], wo[l, d])
                    for c in range(2):
                        nc.tensor.matmul(aop[:, 0:SH], wt[:, c, :],
                                         yt2[c][:, 0:SH],
                                         start=(c == 0), stop=(c == 1))
                    att = gpb.tile([128, SH], BF16, tag="gpb")
                    nc.scalar.copy(att[:], aop[:, 0:SH])
                    nc.gpsimd.dma_start(arb_p[0][h][128 * d:128 * d + 128, :],
                                        att[:])

            def mlp_half(l, h):
                """AR1 consume + MLP for half h -> bf16 bounce for AR2."""
                cs = cs_of(h)
                for d in range(ND):
                    ds = slice(128 * d, 128 * d + 128)
                    att = wop.tile([128, SH], BF16, tag="wop")
                    nc.scalar.dma_start(att[:], arb_o[0][h][ds, :])
                    asc = scal_s[:, d, 5 * l + 2:5 * l + 3]
                    nc.vector.scalar_tensor_tensor(
                        x_t[d][:, cs], att[:], asc, x_t[d][:, cs],
                        AB.mult, AB.add)
                # rmsnorm -> xn (half-width buffers)
                ssq_ps = psB.tile([128, SH], F32, tag="psB")
                ssq_half(ssq_ps, h, x_t, split=True)
                nc.scalar.activation(rl_row[:, cs], ssq_ps[0:1, 0:SH],
                                     AFT.Sqrt,
                                     bias=epsc[0:1, :], scale=1.0 / DM)
                nc.vector.reciprocal_approx_fast(out=rl_row[:, cs],
                                                 in_=rl_row[:, cs])
                nc.scalar.copy(brow[0:1, 0:SH], rl_row[:, cs])
                bcp = psB.tile([128, SH], F32, tag="psB")
                nc.tensor.matmul(bcp[:, 0:SH], onesb[0:1, :], brow[0:1, 0:SH],
                                 start=True, stop=True)
                nc.scalar.copy(bc_sb[:], bcp[:, 0:SH])
                for d in range(ND):
                    nc.vector.tensor_tensor(xn_t[d][:], x_t[d][:, cs],
                                            bc_sb[:], AB.mult)
                # fc + leaky_relu2
                for hh in range(ND):
                    hp = psB.tile([128, SH], F32, tag="psB")
                    wt = wfp.tile([128, ND, 128], F32R, tag="wfp")
                    nc.sync.dma_start(wt[:], wfc[l, hh])
                    for d in range(ND):
                        nc.tensor.matmul(hp[:, 0:SH], wt[:, d, :], xn_t[d][:],
                                         start=(d == 0), stop=(d == ND - 1))
                    hraw = gp.tile([128, SH], F32, tag="gp")
                    nc.scalar.copy(hraw[:], hp[:, 0:SH])
                    hm = gp.tile([128, SH], F32, tag="gp")
                    nc.vector.tensor_scalar(hm[:], hraw[:], 0.0, 0.01,
                                            AB.min, AB.mult)
                    h2 = gp.tile([128, SH], F32, tag="gp")
                    nc.vector.scalar_tensor_tensor(h2[:], hraw[:], 0.0,
                                                   hraw[:], AB.max, AB.mult)
                    nc.vector.scalar_tensor_tensor(h_t[hh][:, cs], h2[:], 1.0,
                                                   hm[:], AB.mult, AB.add)
                # down proj -> bf16 bounce
                for d in range(ND):
                    mlpp = psB.tile([128, SH], F32, tag="psB")
                    wt = wpp.tile([128, ND, 128], BF16, tag="wpp")
                    nc.sync.dma_start(wt[:], wp[l, d])
                    for hh in range(ND):
                        nc.tensor.matmul(mlpp[:, 0:SH], wt[:, hh, :],
                                         h_t[hh][:, cs],
                                         start=(hh == 0), stop=(hh == ND - 1))
                    mt = gpb.tile([128, SH], BF16, tag="gpb")
                    nc.scalar.copy(mt[:], mlpp[:, 0:SH])
                    nc.gpsimd.dma_start(arb_p[1][h][128 * d:128 * d + 128, :],
                                        mt[:])

            def trigger(c, h):
                nc.gpsimd.collective_compute(
                    "AllReduce", AB.add, replica_groups=RG,
                    ins=[arb_p[c][h][:, :].opt()],
                    outs=[arb_o[c][h][:, :].opt()])

            # ================= layer loop =================
            for l in range(L):
                resid_a(l)
                residmix_b(l)
                pss_b = qkv_b_partial(l)
                lif_chain(l, 0, 0)
                consume_b(l, pss_b)
                lif_chain(l, 0, 1)
                lnbc_half(l, 1)
                rope_qk(l, 1)
                lif_chain(l, 1, 0)
                attn_half(l, 0)
                lif_chain(l, 1, 1)
                wo_half(l, 0)
                trigger(0, 0)
                attn_half(l, 1)
                wo_half(l, 1)
                trigger(0, 1)
                mlp_half(l, 0)
                trigger(1, 0)
                mlp_half(l, 1)
                trigger(1, 1)
                if l < L - 1:
                    nc.sync.dma_start(wqkv_s[:], wqkv[l + 1])

            # ---- final: AR2 consume + norm + logits per half ----
            for h in range(2):
                cs = cs_of(h)
                for d in range(ND):
                    ds = slice(128 * d, 128 * d + 128)
                    mt = wop.tile([128, SH], BF16, tag="wop")
                    nc.scalar.dma_start(mt[:], arb_o[1][h][ds, :])
                    msc = scal_s[:, d, 5 * (L - 1) + 3:5 * (L - 1) + 4]
                    nc.vector.scalar_tensor_tensor(
                        x_t[d][:, cs], mt[:], msc, x_t[d][:, cs],
                        AB.mult, AB.add)
                ssq_ps = psB.tile([128, SH], F32, tag="psB")
                ssq_half(ssq_ps, h, x_t, split=True)
                nc.scalar.activation(rl_row[:, cs], ssq_ps[0:1, 0:SH],
                                     AFT.Sqrt,
                                     bias=epsc[0:1, :], scale=1.0 / DM)
                nc.vector.reciprocal_approx_fast(out=rl_row[:, cs],
                                                 in_=rl_row[:, cs])
                nc.scalar.copy(brow[0:1, 0:SH], rl_row[:, cs])
                bcp = psB.tile([128, SH], F32, tag="psB")
                nc.tensor.matmul(bcp[:, 0:SH], onesb[0:1, :], brow[0:1, 0:SH],
                                 start=True, stop=True)
                nc.scalar.copy(bc_sb[:], bcp[:, 0:SH])
                # cast normed x to bf16, reusing dead h_t space
                xnb = [h_t[d][:, 0:SH] for d in range(ND)]
                for d in range(ND):
                    nc.vector.tensor_tensor(xn_t[d][:], x_t[d][:, cs],
                                            bc_sb[:], AB.mult)
                    nc.gpsimd.tensor_copy(xnb[d], xn_t[d][:])
                for o in range(HASH_PC // 512):
                    lg_ps = [psB.tile([128, 512], F32, tag="psB",
                                      name=f"lgp{i}") for i in range(4)]
                    for d in range(ND):
                        ut = unp.tile([128, 512], BF16, tag="unp")
                        qeng = (nc.gpsimd, nc.sync, nc.scalar, nc.sync)[d % 4]
                        qeng.dma_start(ut[:],
                                       unit[128 * d:128 * d + 128,
                                            512 * o:512 * o + 512])
                        for tl in range(4):
                            nc.tensor.matmul(
                                lg_ps[tl][:, 0:512],
                                xnb[d][:, 128 * tl:128 * tl + 128],
                                ut[:], start=(d == 0), stop=(d == ND - 1))
                    for tl in range(4):
                        t = 4 * h + tl
                        ot = gpb.tile([128, 512], BF16, tag="gpb")
                        nc.scalar.copy(ot[:], lg_ps[tl][:, 0:512])
                        (nc.sync if tl % 2 else nc.gpsimd).dma_start(
                            out_lg[128 * t:128 * t + 128,
                                   512 * o:512 * o + 512], ot[:])

    nc.compile()
    return nc


def _host_prep(inputs):
    ids = np.asarray(inputs["input_ids"])
    uni = np.ascontiguousarray(inputs["uni"], np.float32)
    bi = np.ascontiguousarray(inputs["bi"], np.float32)
    Wq = np.asarray(inputs["Wq"], dtype=np.float32)
    Wk = np.asarray(inputs["Wk"], dtype=np.float32)
    Wv = np.asarray(inputs["Wv"], dtype=np.float32)
    Wo = np.asarray(inputs["Wo"], dtype=np.float32)
    Wfc = np.asarray(inputs["Wfc"], dtype=np.float32)
    Wp = np.asarray(inputs["Wp"], dtype=np.float32)
    qg = np.asarray(inputs["q_gain"], dtype=np.float32)
    asc = np.asarray(inputs["attn_scale"], dtype=np.float32)
    msc = np.asarray(inputs["mlp_scale"], dtype=np.float32)
    rmx = np.asarray(inputs["resid_mix"], dtype=np.float32)

    prev = np.concatenate([np.zeros_like(ids[:, :1]), ids[:, :-1]], axis=1)
    h1 = (ids % HASH).astype(np.int64)
    h2 = ((prev.astype(np.int64) * 31 + ids) % HASH).astype(np.int64)

    inv_freq = 1.0 / (ROPE_BASE ** (np.arange(0, DH, 2, dtype=np.float32) / DH))
    freqs = np.arange(S, dtype=np.float32)[:, None] * inv_freq[None, :]
    cos = np.cos(freqs).astype(np.float32)   # [S, 32]
    sin = np.sin(freqs).astype(np.float32)
    cos64 = np.ascontiguousarray(np.concatenate([cos, cos], axis=1).T)  # [64,S]
    sin64 = np.ascontiguousarray(np.concatenate([sin, -sin], axis=1).T)
    cosq = np.ascontiguousarray(np.tile(cos64, (2, 1)))   # [128, S]
    sinq = np.ascontiguousarray(np.tile(sin64, (2, 1)))

    # swap permutations: P~[k, m] = 1 iff k = partner(m) (partner: +-32 in 64)
    pswp = np.zeros((128, 2, 128), np.float32)
    for m in range(128):
        base = (m // 64) * 64
        partner = base + (m % 64 + 32) % 64
        pswp[partner, 0, m] = 1.0
    for m in range(64):
        pswp[(m + 32) % 64, 1, m] = 1.0
    ident = np.eye(128, dtype=np.float32)
    ident[64:128, 0:64] += np.eye(64, dtype=np.float32)
    trim = np.tril(np.ones((128, 128), np.float32)).T.copy()
    sel2f = np.zeros((2, 128), np.float32)
    sel2f[0, 0:64] = 1.0
    sel2f[1, 64:128] = 1.0
    selT = np.ascontiguousarray(sel2f.T)

    # scal columns: rm0, rm1, attn_scale, mlp_scale, rm0*msc_prev
    scal = np.zeros((128, ND, 5 * L), np.float32)
    for l in range(L):
        fold = rmx[l, 0] * (msc[l - 1] if l > 0 else 0.0)
        for v, vec in enumerate((rmx[l, 0], rmx[l, 1], asc[l], msc[l], fold)):
            scal[:, :, 5 * l + v] = np.asarray(vec).reshape(ND, 128).T

    in_maps = []
    for core in range(N_CORES):
        g, r = core // TP, core % TP
        qsl = slice(QD * r, QD * (r + 1))
        ksl = slice(KD * r, KD * (r + 1))
        hsl = slice(HID_PC * r, HID_PC * (r + 1))
        asl = slice(HASH_PC * r, HASH_PC * (r + 1))
        wqkv = np.concatenate([
            Wq[:, qsl, :].transpose(0, 2, 1),
            Wk[:, ksl, :].transpose(0, 2, 1),
            Wv[:, ksl, :].transpose(0, 2, 1)], axis=2)  # [L, DM, 384]
        wqkv_t = np.ascontiguousarray(
            wqkv.reshape(L, ND, 128, QD + 2 * KD).transpose(0, 2, 1, 3))
        woT = Wo[:, :, qsl].transpose(0, 2, 1)          # [L, 256, DM]
        wo_t = np.ascontiguousarray(
            woT.reshape(L, 2, 128, ND, 128).transpose(0, 3, 2, 1, 4))
        wfcT = Wfc[:, hsl, :].transpose(0, 2, 1)        # [L, DM, HID_PC]
        wfc_t = np.ascontiguousarray(
            wfcT.reshape(L, ND, 128, ND, 128).transpose(0, 3, 2, 1, 4))
        wpT = Wp[:, :, hsl].transpose(0, 2, 1)          # [L, HID_PC, DM]
        wp_t = np.ascontiguousarray(
            wpT.reshape(L, ND, 128, ND, 128).transpose(0, 3, 2, 1, 4)
        ).astype(BF16_NP)
        qgain = np.zeros((128, 2, L), np.float32)
        for l in range(L):
            for j in range(2):
                for hp in range(2):
                    head = HEADS_PC * r + 2 * j + hp
                    qgain[64 * hp:64 * hp + 64, j, l] = qg[l, head]
        m = dict(
            xe1=np.ascontiguousarray(uni[h1[g]].T),
            xe2=np.ascontiguousarray(bi[h2[g]].T),
            wqkv=wqkv_t,
            wo=wo_t,
            wfc=wfc_t,
            wp=wp_t,
            unit=np.ascontiguousarray(uni[asl, :].T).astype(BF16_NP),
            cosq=cosq,
            sinq=sinq,
            cosk=cos64,
            sink=sin64,
            pswp=pswp,
            ident=ident,
            tri=trim,
            selT=selT,
            sel2f=sel2f,
            scal=scal,
            qgain=qgain,
        )
        in_maps.append(m)
    return in_maps


def kernel(**inputs):
    if "nc" not in _CACHE:
        _CACHE["nc"] = build_program()
    nc = _CACHE["nc"]
    in_maps = _host_prep(inputs)
    res = run_bass_kernel_spmd(nc, in_maps, core_ids=list(range(N_CORES)),
                               trace=os.environ.get("K_TRACE", "0") == "1")
    _CACHE["res"] = res
    out = np.zeros((B, S, HASH), np.float32)
    for core in range(N_CORES):
        g, r = core // TP, core % TP
        out[g, :, HASH_PC * r:HASH_PC * (r + 1)] = res.results[core]["out_lg"]
    return out


# revision 55
# speedup vs baseline: 1.0130x; 1.0130x over previous
"""Trainium2 Bass kernel for nn_GPT_61409442398424 (4-layer spiking GPT).

Sharding: DP-2 over batch (core groups {0-3},{4-7}) x TP-4 within group
(Wq/Wk/Wv by heads, Wfc/Wp by hidden dim, uni by HASH rows for logits).

v3 design notes (on top of v2), measured 2435us -> 1623us:
- Token-halved pipeline: every layer phase (QKV/rope/LIF/attention/Wo/
  AllReduce/MLP) runs per 512-token half so each AllReduce overlaps the
  other half's compute. ARs stay 1MB bf16 (mesh regime, ~33us each).
- LIF fixpoint (KFIX=7, adds ~7e-3 rel err; K=6 would breach the 2e-2
  gate) runs per half with carry chaining via tensor_tensor_scan's AP
  `initial`. Scans/STT are DVE-only (GpSimd lacks both; its
  tensor_scalar ucode is ~7us per [128,512] op - avoid).
- Half-b QKV is precomputed against pre-AllReduce x during the fix_a
  window, then a delta pass folds in rm0*msc*AR2b after it lands, so the
  post-AR critical path only carries 24 delta matmuls + rope.
- Attention software-pipelined: exp'd score tiles produced LOOK=3 tiles
  ahead of the PV matmuls, heads interleaved; the causal tri mask runs
  on GpSimd so attention never queues behind the other half's LIF.
- Weights host-packed into exact tile layouts (>=1KB contiguous per
  partition per descriptor); wp/unit/h/logits in bf16; the q-rms,
  attention-denominator, and norm-row partition-broadcast matmuls are
  single bf16/f32 [2,SH]-row matmuls instead of pairs of 4cyc/row f32.
- AR-consume tiles use a separate pool from outbound evac tiles and
  their DMAs issue from the Scalar queue; outbound bf16 evacs DMA from
  the GpSimd queue (a dma_start costs ~0.6us of its queue).
- PE runs at K=4/8 (1.2GHz) for ~60% of the span due to the HAM activity
  throttle + board GPIO caps; dense 12us+ matmul runs still measure
  cold, so further scheduling densification has limited return.
"""
import os
import numpy as np

import concourse.bass as bass
import concourse.tile as tile
from concourse import bacc, mybir
from concourse.bass_utils import run_bass_kernel_spmd

F32 = mybir.dt.float32
F32R = mybir.dt.float32r
BF16 = mybir.dt.bfloat16
FP16 = mybir.dt.float16
AB = mybir.AluOpType
AFT = mybir.ActivationFunctionType
BF16_NP = mybir.dt.np(mybir.dt.bfloat16)

B, S, DM, H, HKV, L, MLP_MULT = 2, 1024, 1024, 16, 4, 4, 4
DH = DM // H
HASH, VOCAB = 16384, 50257
EPS = 1.1920929e-07
THRESH, DECAY = 0.8, 0.9
ROPE_BASE = 10000.0
N_CORES = 8
TP = 4
HEADS_PC = H // TP        # 4 q heads per core
QD = HEADS_PC * DH        # 256 q dims per core
KD = DH                   # 64 kv dims per core (1 kv head)
HID_PC = MLP_MULT * DM // TP
HASH_PC = HASH // TP
NT = S // 128
ND = DM // 128
SH = S // 2               # tokens per half
KFIX = 7                  # LIF fixpoint scans per half

_CACHE = {}


def build_program():
    nc = bacc.Bacc("TRN2", target_bir_lowering=False, debug=False,
                   enable_asserts=False, num_devices=N_CORES)

    din = {}
    def di(name, shape, dt=F32R):
        din[name] = nc.dram_tensor(name, shape, dt, kind="ExternalInput").ap()
        return din[name]

    xe1 = di("xe1", [DM, S], F32R)
    xe2 = di("xe2", [DM, S], F32R)
    wqkv = di("wqkv", [L, 128, ND, QD + 2 * KD], F32R)  # tile-packed
    wo = di("wo", [L, ND, 128, 2, 128], F32R)           # [l,d,p,c,f]
    wfc = di("wfc", [L, ND, 128, ND, 128], F32R)        # [l,hh,p,d,f]
    wp = di("wp", [L, ND, 128, ND, 128], BF16)          # [l,d,p,hh,f]
    unit = di("unit", [DM, HASH_PC], BF16)
    cosq = di("cosq", [128, S], F32)     # q-tile rope tables (2 heads/tile)
    sinq = di("sinq", [128, S], F32)     # signed
    cosk = di("cosk", [64, S], F32)
    sink = di("sink", [64, S], F32)
    pswp = di("pswp", [128, 2, 128], F32R)  # [:,0,:]=Pq ; [0:64,1,0:64]=Pk
    ident = di("ident", [128, 128], F32R)
    tri = di("tri", [128, 128], F32R)
    selT = di("selT", [128, 2], F32R)
    sel2f = di("sel2f", [2, 128], FP16)
    scal = di("scal", [128, ND, 5 * L], F32)
    qgain = di("qgain", [128, 2, L], F32)
    out_lg = nc.dram_tensor("out_lg", [S, HASH_PC], BF16, kind="ExternalOutput").ap()

    # ---------------- persistent SBUF ------------------------------------
    x_t = [nc.alloc_sbuf_tensor(f"x_{d}", [128, S], F32R) for d in range(ND)]
    xn_t = [nc.alloc_sbuf_tensor(f"xn_{d}", [128, SH], F32R) for d in range(ND)]
    h_t = [nc.alloc_sbuf_tensor(f"h_{d}", [128, S], BF16) for d in range(ND)]
    qsb = [nc.alloc_sbuf_tensor(f"qsb_{j}", [128, S], F32R) for j in range(2)]
    kvsb = nc.alloc_sbuf_tensor("kvsb", [128, SH], F32R)
    q4 = [nc.alloc_sbuf_tensor(f"q4_{j}", [128, S], F32R) for j in range(2)]
    u2 = [nc.alloc_sbuf_tensor(f"u2_{j}", [128, S], F32) for j in range(2)]
    c2 = [nc.alloc_sbuf_tensor(f"c2_{j}", [128, S + 1], F32) for j in range(2)]
    e2 = [nc.alloc_sbuf_tensor(f"e2_{j}", [128, SH], F32) for j in range(2)]
    et2 = [nc.alloc_sbuf_tensor(f"et2_{j}", [128, SH], F32) for j in range(2)]
    yt2 = [nc.alloc_sbuf_tensor(f"yt2_{j}", [128, SH], F32R) for j in range(2)]
    v65 = nc.alloc_sbuf_tensor("v65", [128, NT, 65], F32R)
    kt2 = nc.alloc_sbuf_tensor("kt2", [128, S], F32R)
    bc_sb = nc.alloc_sbuf_tensor("bc_sb", [128, SH], F32)
    wqkv_s = nc.alloc_sbuf_tensor("wqkv_s", [128, ND, QD + 2 * KD], F32R)
    cosq_s = nc.alloc_sbuf_tensor("cosq_s", [128, S], F32)
    sinq_s = nc.alloc_sbuf_tensor("sinq_s", [128, S], F32)
    cosk_s = nc.alloc_sbuf_tensor("cosk_s", [64, S], F32)
    sink_s = nc.alloc_sbuf_tensor("sink_s", [64, S], F32)
    pswp_s = nc.alloc_sbuf_tensor("pswp_s", [128, 2, 128], F32R)
    ident_s = nc.alloc_sbuf_tensor("ident_s", [128, 128], F32R)
    tri_s = nc.alloc_sbuf_tensor("tri_s", [128, 128], F32R)
    scal_s = nc.alloc_sbuf_tensor("scal_s", [128, ND, 5 * L], F32)
    qgain_s = nc.alloc_sbuf_tensor("qgain_s", [128, 2, L], F32)
    rkc = nc.alloc_sbuf_tensor("rkc", [128, NT], F32)    # 0.125/rms(k) per key
    lnbc = nc.alloc_sbuf_tensor("lnbc", [128, NT], F32)  # ln(bc) per key
    ibc = nc.alloc_sbuf_tensor("ibc", [128, NT], F32R)   # 1/bc per key
    rows_sb = nc.alloc_sbuf_tensor("rows_sb", [128, S], F32)
    hrows = nc.alloc_sbuf_tensor("hrows", [2, SH], FP16)
    drows = nc.alloc_sbuf_tensor("drows", [33, SH], BF16)
    selT_s = nc.alloc_sbuf_tensor("selT_s", [128, 2], F32R)
    sel2f_s = nc.alloc_sbuf_tensor("sel2f_s", [2, 128], FP16)
    onesb = nc.alloc_sbuf_tensor("onesb", [33, 128], BF16)
    brow = nc.alloc_sbuf_tensor("brow", [1, SH], BF16)
    onesr_f = nc.alloc_sbuf_tensor("onesr_f", [128, 128], F32)
    onesr = nc.alloc_sbuf_tensor("onesr", [128, 128], F32R)
    onesc_f = nc.alloc_sbuf_tensor("onesc_f", [128, 1], F32)
    onesc = nc.alloc_sbuf_tensor("onesc", [128, 1], F32R)
    d9_s = nc.alloc_sbuf_tensor("d9_s", [128, 1], F32)
    mtmp = nc.alloc_sbuf_tensor("mtmp", [128, 1], F32)
    zc = nc.alloc_sbuf_tensor("zc", [128, 1], F32)
    epsc = nc.alloc_sbuf_tensor("epsc", [128, 1], F32)
    rl_row = rows_sb[0:1, :]         # 1/rms per token (norms)
    ln_row = rows_sb[64:65, :]       # ln per token (v bias)

    RG = [[0, 1, 2, 3], [4, 5, 6, 7]]

    with tile.TileContext(nc) as tc:
        with tc.tile_pool(name="gp", bufs=4) as gp, \
             tc.tile_pool(name="gpb", bufs=3) as gpb, \
             tc.tile_pool(name="etp", bufs=4) as etp, \
             tc.tile_pool(name="wop", bufs=3) as wop, \
             tc.tile_pool(name="wfp", bufs=3) as wfp, \
             tc.tile_pool(name="wpp", bufs=3) as wpp, \
             tc.tile_pool(name="unp", bufs=8) as unp, \
             tc.tile_pool(name="psB", bufs=6, space="PSUM") as psB, \
             tc.tile_pool(name="psY", bufs=2, space="PSUM") as psY, \
             tc.tile_pool(name="dram", bufs=1, space="DRAM") as dram:

            arb_i = dram.tile([DM, S], F32R)
            arb_p = [[dram.tile([DM, SH], BF16, name=f"arbp{c}{h}")
                      for h in range(2)] for c in range(2)]
            arb_o = [[dram.tile([DM, SH], BF16, name=f"arbo{c}{h}")
                      for h in range(2)] for c in range(2)]

            # ---- constants / tables ----
            nc.sync.dma_start(cosq_s[:], cosq[:])
            nc.sync.dma_start(sinq_s[:], sinq[:])
            nc.sync.dma_start(cosk_s[:], cosk[:])
            nc.sync.dma_start(sink_s[:], sink[:])
            nc.sync.dma_start(pswp_s[:], pswp[:])
            nc.sync.dma_start(ident_s[:], ident[:])
            nc.sync.dma_start(tri_s[:], tri[:])
            nc.sync.dma_start(selT_s[:], selT[:])
            nc.sync.dma_start(sel2f_s[:], sel2f[:])
            nc.sync.dma_start(scal_s[:], scal[:])
            nc.sync.dma_start(qgain_s[:], qgain[:])
            nc.sync.dma_start(wqkv_s[:], wqkv[0])
            nc.vector.memset(mtmp[:], 1.0)
            nc.vector.tensor_copy(onesc[:], mtmp[:])
            nc.vector.tensor_copy(onesr[:], mtmp[:, 0:1].to_broadcast((128, 128)))
            nc.vector.tensor_copy(onesr_f[:], mtmp[:, 0:1].to_broadcast((128, 128)))
            nc.vector.tensor_copy(onesb[:], mtmp[0:33, 0:1].to_broadcast((33, 128)))
            nc.vector.tensor_copy(onesc_f[:], mtmp[:])
            nc.vector.memset(d9_s[:], 0.9)
            nc.vector.memset(zc[:], 0.0)
            nc.vector.memset(epsc[:], EPS)

            # ---- embedding: x = xe1 + xe2 (also x0, kept in DRAM) ----
            for d in range(ND):
                ds = slice(128 * d, 128 * d + 128)
                nc.sync.dma_start(x_t[d][:], xe2[ds, :])
                for eh in range(2):
                    ecs = slice(SH * eh, SH * eh + SH)
                    t1 = gp.tile([128, SH], F32R, tag="gp")
                    nc.sync.dma_start(t1[:], xe1[ds, ecs])
                    nc.gpsimd.tensor_tensor(x_t[d][:, ecs], x_t[d][:, ecs],
                                            t1[:], AB.add)
                    nc.sync.dma_start(arb_i[ds, ecs], x_t[d][:, ecs])

            def cs_of(h):
                return slice(SH * h, SH * h + SH)

            def ssq_half(ps, h, src, split=False):
                """ps[0:1, 0:SH] = sum over DM of src^2 for token half h."""
                for d in range(ND):
                    sq = gp.tile([128, SH], F32R, tag="gp")
                    if split and d % 2 == 1:
                        nc.vector.tensor_tensor(sq[:], src[d][:, cs_of(h)],
                                                src[d][:, cs_of(h)], AB.mult)
                    else:
                        nc.scalar.activation(sq[:], src[d][:, cs_of(h)],
                                             AFT.Square, bias=zc[:])
                    nc.tensor.matmul(ps[0:1, 0:SH], onesc[:], sq[:],
                                     start=(d == 0), stop=(d == ND - 1))

            def lnbc_half(l, h):
                """Per-token ln(rsqrt(mean x^2+eps)) for v (exp bias), half h."""
                cs = cs_of(h)
                ssq_ps = psB.tile([128, SH], F32, tag="psB")
                ssq_half(ssq_ps, h, x_t)
                nc.scalar.activation(ln_row[:, cs], ssq_ps[0:1, 0:SH], AFT.Ln,
                                     bias=epsc[0:1, :], scale=1.0 / DM)
                nc.vector.tensor_scalar(ln_row[:, cs], ln_row[:, cs], -0.5,
                                        None, AB.mult)
                lnp = psB.tile([128, SH], F32, tag="psB")
                for tl in range(4):
                    t = 4 * h + tl
                    nc.tensor.transpose(lnp[:, tl:tl + 1],
                                        rows_sb[64:65, 128 * t:128 * t + 128]
                                        .bitcast(F32),
                                        ident_s[64:65, 64:65].bitcast(F32))
                nc.scalar.copy(lnbc[:, 4 * h:4 * h + 4], lnp[:, 0:4])
                nc.scalar.activation(ibc[:, 4 * h:4 * h + 4],
                                     lnbc[:, 4 * h:4 * h + 4], AFT.Exp,
                                     bias=zc[:], scale=-1.0)

            def rope_qk(l, h):
                """q-head rms + rope, k rope + rms, v transpose for half h."""
                cs = cs_of(h)
                for jt in range(2):
                    sq = gp.tile([128, SH], F32R, tag="gp")
                    nc.scalar.activation(sq[:], qsb[jt][:, cs], AFT.Square,
                                         bias=zc[:])
                    rq_ps = psB.tile([128, SH], F32, tag="psB")
                    nc.tensor.matmul(rq_ps[0:2, 0:SH], selT_s[:], sq[:],
                                     start=True, stop=True)
                    nc.scalar.activation(hrows[0:2, 0:SH], rq_ps[0:2, 0:SH],
                                         AFT.Sqrt, bias=zc[0:2, :],
                                         scale=1.0 / DH)
                    rqb = psB.tile([128, SH], F32, tag="psB")
                    nc.tensor.matmul(rqb[:, 0:SH], sel2f_s[:],
                                     hrows[0:2, 0:SH], start=True, stop=True)
                    rqi = gp.tile([128, SH], F32, tag="gp", name="rqi")
                    nc.vector.reciprocal_approx_fast(out=rqi[:],
                                                     in_=rqb[:, 0:SH])
                    swp = psB.tile([128, SH], F32, tag="psB")
                    nc.tensor.matmul(swp[:, 0:SH], pswp_s[:, 0, :],
                                     qsb[jt][:, cs], start=True, stop=True)
                    t1 = et2[0][:, 0:SH]
                    nc.vector.scalar_tensor_tensor(
                        t1, qsb[jt][:, cs], 1.0, cosq_s[:, cs],
                        AB.mult, AB.mult)
                    t2 = et2[1][:, 0:SH]
                    nc.vector.scalar_tensor_tensor(
                        t2, swp[:, 0:SH], 1.0, sinq_s[:, cs],
                        AB.mult, AB.mult)
                    nc.vector.scalar_tensor_tensor(
                        t1, t1, 1.0, t2, AB.mult, AB.add)
                    nc.vector.scalar_tensor_tensor(
                        qsb[jt][:, cs], t1, 1.0, rqi[:], AB.mult, AB.mult)

                # k rope
                swp = psB.tile([128, SH], F32, tag="psB")
                nc.tensor.matmul(swp[0:64, 0:SH], pswp_s[0:64, 1, 0:64],
                                 kvsb[0:64, 0:SH], start=True, stop=True)
                t1 = et2[0][0:64, 0:SH]
                nc.vector.scalar_tensor_tensor(
                    t1, kvsb[0:64, 0:SH], 1.0, cosk_s[:, cs],
                    AB.mult, AB.mult)
                t2 = et2[1][0:64, 0:SH]
                nc.vector.scalar_tensor_tensor(
                    t2, swp[0:64, 0:SH], 1.0, sink_s[:, cs],
                    AB.mult, AB.mult)
                nc.vector.scalar_tensor_tensor(
                    kt2[0:64, cs], t1, 1.0, t2, AB.mult, AB.add)
                nc.scalar.copy(kt2[64:128, cs], kt2[0:64, cs])

                # k-head rms -> per-key scale column (0.125/rms)
                ksq = gp.tile([128, SH], F32, tag="gp")
                nc.scalar.activation(ksq[0:64, :], kt2[0:64, cs], AFT.Square,
                                     bias=zc[0:64, :])
                rkp = psB.tile([128, SH], F32, tag="psB")
                for tl in range(4):
                    nc.tensor.matmul(rkp[:, tl:tl + 1],
                                     ksq[0:64, 128 * tl:128 * tl + 128],
                                     onesc_f[0:64, :],
                                     start=True, stop=True)
                tsl = slice(4 * h, 4 * h + 4)
                nc.scalar.activation(rkc[:, tsl], rkp[:, 0:4], AFT.Sqrt,
                                     bias=zc[:], scale=1.0 / DH)
                nc.vector.reciprocal_approx_fast(out=rkc[:, tsl],
                                                 in_=rkc[:, tsl])
                nc.vector.tensor_scalar(rkc[:, tsl], rkc[:, tsl], 0.125,
                                        None, AB.mult)
                # v -> token-major tiles via PE transpose
                for tl in range(4):
                    t = 4 * h + tl
                    vtp = psB.tile([128, SH], F32, tag="psB")
                    nc.tensor.transpose(vtp[:, 0:64].bitcast(F32R),
                                        kvsb[64:128, 128 * tl:128 * tl + 128],
                                        ident_s[64:128, 0:64])
                    nc.scalar.copy(v65[:, t, 0:64], vtp[:, 0:64])
                    nc.gpsimd.tensor_copy(v65[:, t, 64:65], ibc[:, t:t + 1])

            def resid_a(l):
                """AR2a consume + resid mix + QKV + lnbc + rope for half a."""
                cs = cs_of(0)
                for d in range(ND):
                    ds = slice(128 * d, 128 * d + 128)
                    rm0 = scal_s[:, d, 5 * l + 0:5 * l + 1]
                    rm1 = scal_s[:, d, 5 * l + 1:5 * l + 2]
                    if l == 0:
                        tt = gp.tile([128, SH], F32, tag="gp")
                        nc.scalar.mul(tt[:], x_t[d][:, cs], rm1)
                        nc.vector.scalar_tensor_tensor(
                            x_t[d][:, cs], x_t[d][:, cs], rm0, tt[:],
                            AB.mult, AB.add)
                    else:
                        x0t = gp.tile([128, SH], F32R, tag="gp")
                        nc.sync.dma_start(x0t[:], arb_i[ds, cs])
                        tt = gp.tile([128, SH], F32, tag="gp")
                        nc.scalar.mul(tt[:], x0t[:], rm1)
                        nc.vector.scalar_tensor_tensor(
                            x_t[d][:, cs], x_t[d][:, cs], rm0, tt[:],
                            AB.mult, AB.add)
                        art = wop.tile([128, SH], BF16, tag="wop")
                        nc.scalar.dma_start(art[:], arb_o[1][0][ds, :])
                        fold = scal_s[:, d, 5 * l + 4:5 * l + 5]
                        nc.vector.scalar_tensor_tensor(
                            x_t[d][:, cs], art[:], fold, x_t[d][:, cs],
                            AB.mult, AB.add)
                pss = [psB.tile([128, SH], F32, tag="psB", name=f"qkva{i}")
                       for i in range(3)]
                for d in range(ND):
                    for jt in range(3):
                        nc.tensor.matmul(
                            pss[jt][:, 0:SH],
                            wqkv_s[:, d, 128 * jt:128 * jt + 128],
                            x_t[d][:, cs],
                            start=(d == 0), stop=(d == ND - 1))
                for jt in range(2):
                    nc.scalar.copy(qsb[jt][:, cs], pss[jt][:, 0:SH])
                nc.scalar.copy(kvsb[:, 0:SH], pss[2][:, 0:SH])
                lnbc_half(l, 0)
                rope_qk(l, 0)

            def residmix_b(l):
                """x_b = rm0*x_b + rm1*x0_b (AR2b part folded in later)."""
                cs = cs_of(1)
                for d in range(ND):
                    ds = slice(128 * d, 128 * d + 128)
                    rm0 = scal_s[:, d, 5 * l + 0:5 * l + 1]
                    rm1 = scal_s[:, d, 5 * l + 1:5 * l + 2]
                    if l == 0:
                        tt = gp.tile([128, SH], F32, tag="gp")
                        nc.scalar.mul(tt[:], x_t[d][:, cs], rm1)
                        nc.vector.scalar_tensor_tensor(
                            x_t[d][:, cs], x_t[d][:, cs], rm0, tt[:],
                            AB.mult, AB.add)
                    else:
                        x0t = gp.tile([128, SH], F32R, tag="gp")
                        nc.sync.dma_start(x0t[:], arb_i[ds, cs])
                        tt = gp.tile([128, SH], F32, tag="gp")
                        nc.scalar.mul(tt[:], x0t[:], rm1)
                        nc.vector.scalar_tensor_tensor(
                            x_t[d][:, cs], x_t[d][:, cs], rm0, tt[:],
                            AB.mult, AB.add)

            def qkv_b_partial(l):
                """QKV over pre-AR x_b; for l=0 this is the whole thing."""
                cs = cs_of(1)
                pss = [psB.tile([128, SH], F32, tag="psB", name=f"qkvb{i}")
                       for i in range(3)]
                for d in range(ND):
                    for jt in range(3):
                        nc.tensor.matmul(
                            pss[jt][:, 0:SH],
                            wqkv_s[:, d, 128 * jt:128 * jt + 128],
                            x_t[d][:, cs],
                            start=(d == 0),
                            stop=(l == 0 and d == ND - 1))
                if l == 0:
                    for jt in range(2):
                        nc.scalar.copy(qsb[jt][:, cs], pss[jt][:, 0:SH])
                    nc.scalar.copy(kvsb[:, 0:SH], pss[2][:, 0:SH])
                return pss

            def consume_b(l, pss):
                """Fold AR2b into x_b and into the QKV_b psums (delta pass)."""
                if l == 0:
                    return
                cs = cs_of(1)
                for d in range(ND):
                    ds = slice(128 * d, 128 * d + 128)
                    art = wop.tile([128, SH], BF16, tag="wop")
                    nc.scalar.dma_start(art[:], arb_o[1][1][ds, :])
                    fold = scal_s[:, d, 5 * l + 4:5 * l + 5]
                    tmp = gp.tile([128, SH], F32R, tag="gp", name="artmp")
                    nc.scalar.mul(tmp[:], art[:], fold)
                    nc.vector.tensor_tensor(x_t[d][:, cs], x_t[d][:, cs],
                                            tmp[:], AB.add)
                    for jt in range(3):
                        nc.tensor.matmul(
                            pss[jt][:, 0:SH],
                            wqkv_s[:, d, 128 * jt:128 * jt + 128],
                            tmp[:],
                            start=False, stop=(d == ND - 1))
                for jt in range(2):
                    nc.scalar.copy(qsb[jt][:, cs], pss[jt][:, 0:SH])
                nc.scalar.copy(kvsb[:, 0:SH], pss[2][:, 0:SH])

            def lif_chain(l, h, j):
                """LIF fixpoint for half h, chain j (DVE)."""
                cs = cs_of(h)
                h0, h1 = SH * h, SH * h + SH
                d9 = d9_s[:].to_broadcast((128, SH))
                z9 = zc[:].to_broadcast((128, SH))
                if True:
                    eng = nc.vector
                    if h == 0:
                        eng.tensor_tensor_scan(u2[j][:, cs], d9, qsb[j][:, cs],
                                               0.0, AB.mult, AB.add)
                        eng.memset(c2[j][:, 0:1], 0.0)
                    else:
                        eng.tensor_tensor_scan(u2[j][:, cs], d9, qsb[j][:, cs],
                                               u2[j][:, h0 - 1:h0],
                                               AB.mult, AB.add)
                        # decay-only carry fill of c cols before iteration 0
                        eng.tensor_tensor_scan(c2[j][:, h0 + 1:h1 + 1], d9, z9,
                                               c2[j][:, h0:h0 + 1],
                                               AB.mult, AB.max)
                    for p in range(KFIX):
                        if p == 0 and h == 0:
                            eng.scalar_tensor_tensor(
                                e2[j][:], u2[j][:, cs], THRESH, u2[j][:, cs],
                                AB.is_ge, AB.mult)
                        else:
                            eng.scalar_tensor_tensor(
                                e2[j][:], c2[j][:, h0:h1], -DECAY,
                                u2[j][:, cs], AB.mult, AB.add)
                            eng.scalar_tensor_tensor(
                                e2[j][:], e2[j][:], THRESH, u2[j][:, cs],
                                AB.is_ge, AB.mult)
                        init = 0.0 if h == 0 else c2[j][:, h0:h0 + 1]
                        eng.tensor_tensor_scan(
                            c2[j][:, h0 + 1:h1 + 1], d9, e2[j][:], init,
                            AB.mult, AB.max)
                    # final spikes*gain; gated q -> q4
                    eng.scalar_tensor_tensor(
                        e2[j][:], c2[j][:, h0:h1], -DECAY, u2[j][:, cs],
                        AB.mult, AB.add)
                    eng.tensor_scalar(e2[j][:], e2[j][:], THRESH,
                                      qgain_s[:, j, l:l + 1],
                                      AB.is_ge, AB.mult)
                    eng.scalar_tensor_tensor(q4[j][:, cs], qsb[j][:, cs], 1.0,
                                             e2[j][:], AB.mult, AB.mult)

            def attn_half(l, h):
                """Attention for token half h (keys 0..(h+1)*SH), both chains.

                Software-pipelined: exp'd score tiles (et) are produced LOOK
                tiles ahead of the PV matmuls that consume them, with the two
                heads interleaved, so the PE never stalls on Exp latency.
                The causal tri mask runs on GpSimd (DVE is busy with the
                other half's LIF fixpoint)."""
                cs = cs_of(h)
                tlist = list(range(4 * (h + 1)))
                nmm = len(tlist)
                for j in range(2):
                    yups = [psY.tile([128, SH], F32, tag="psY", name=f"yup{i}")
                            for i in range(2)]
                    work = [(hl, t) for t in tlist for hl in range(2)]
                    ets = {}

                    def issue_score(idx):
                        hl, t = work[idx]
                        off = 64 * hl
                        tok0 = max(SH * h, 128 * t)
                        ncols = SH * h + SH - tok0
                        et = etp.tile([128, SH], F32R, tag="etp")
                        scp = psB.tile([128, SH], F32, tag="psB")
                        nc.tensor.matmul(
                            scp[:, 0:ncols],
                            kt2[off:off + 64, 128 * t:128 * t + 128],
                            q4[j][off:off + 64, tok0:SH * h + SH],
                            start=True, stop=True)
                        nc.scalar.activation(
                            et[:, 0:ncols], scp[:, 0:ncols],
                            AFT.Exp, bias=lnbc[:, t:t + 1],
                            scale=rkc[:, t:t + 1])
                        if 128 * t >= SH * h:
                            nc.gpsimd.tensor_tensor(
                                et[:, 0:128], et[:, 0:128], tri_s[:],
                                AB.mult)
                        ets[idx] = et

                    LOOK = 3
                    for idx in range(min(LOOK, len(work))):
                        issue_score(idx)
                    for idx, (hl, t) in enumerate(work):
                        if idx + LOOK < len(work):
                            issue_score(idx + LOOK)
                        et = ets.pop(idx)
                        tok0 = max(SH * h, 128 * t)
                        ncols = SH * h + SH - tok0
                        cols0 = tok0 - SH * h
                        i = tlist.index(t)
                        nc.tensor.matmul(yups[hl][0:65, cols0:SH],
                                         v65[:, t, :], et[:, 0:ncols],
                                         start=(i == 0), stop=(i == nmm - 1))
                    for hl in range(2):
                        off = 64 * hl
                        yup = yups[hl]
                        nc.scalar.copy(q4[j][off:off + 64, cs], yup[0:64, :])
                        nc.scalar.copy(drows[32 * hl:32 * hl + 1, 0:SH],
                                       yup[64:65, :])
                    # epilogue: divide by denominator broadcast
                    rbp = psB.tile([128, SH], F32, tag="psB")
                    for hl in range(2):
                        nc.tensor.matmul(rbp[64 * hl:64 * hl + 64, 0:SH],
                                         onesb[32 * hl:32 * hl + 1, 0:64],
                                         drows[32 * hl:32 * hl + 1, 0:SH],
                                         start=True, stop=True)
                    rbi = gp.tile([128, SH], F32, tag="gp", name="rbi")
                    nc.vector.reciprocal_approx_fast(out=rbi[:], in_=rbp[:, 0:SH])
                    nc.vector.scalar_tensor_tensor(yt2[j][:, 0:SH],
                                                   q4[j][:, cs], 1.0,
                                                   rbi[:], AB.mult, AB.mult)

            def wo_half(l, h):
                """Wo partials for half h -> bf16 bounce; caller triggers AR."""
                cs = cs_of(h)
                for d in range(ND):
                    aop = psB.tile([128, SH], F32, tag="psB")
                    wt = wop.tile([128, 2, 128], F32R, tag="wop")
                    nc.sync.dma_start(wt[:], wo[l, d])
                    for c in range(2):
                        nc.tensor.matmul(aop[:, 0:SH], wt[:, c, :],
                                         yt2[c][:, 0:SH],
                                         start=(c == 0), stop=(c == 1))
                    att = gpb.tile([128, SH], BF16, tag="gpb")
                    nc.scalar.copy(att[:], aop[:, 0:SH])
                    nc.gpsimd.dma_start(arb_p[0][h][128 * d:128 * d + 128, :],
                                        att[:])

            def mlp_half(l, h):
                """AR1 consume + MLP for half h -> bf16 bounce for AR2."""
                cs = cs_of(h)
                for d in range(ND):
                    ds = slice(128 * d, 128 * d + 128)
                    att = wop.tile([128, SH], BF16, tag="wop")
                    nc.scalar.dma_start(att[:], arb_o[0][h][ds, :])
                    asc = scal_s[:, d, 5 * l + 2:5 * l + 3]
                    nc.vector.scalar_tensor_tensor(
                        x_t[d][:, cs], att[:], asc, x_t[d][:, cs],
                        AB.mult, AB.add)
                # rmsnorm -> xn (half-width buffers)
                ssq_ps = psB.tile([128, SH], F32, tag="psB")
                ssq_half(ssq_ps, h, x_t, split=True)
                nc.scalar.activation(rl_row[:, cs], ssq_ps[0:1, 0:SH],
                                     AFT.Sqrt,
                                     bias=epsc[0:1, :], scale=1.0 / DM)
                nc.vector.reciprocal_approx_fast(out=rl_row[:, cs],
                                                 in_=rl_row[:, cs])
                nc.scalar.copy(brow[0:1, 0:SH], rl_row[:, cs])
                bcp = psB.tile([128, SH], F32, tag="psB")
                nc.tensor.matmul(bcp[:, 0:SH], onesb[0:1, :], brow[0:1, 0:SH],
                                 start=True, stop=True)
                nc.scalar.copy(bc_sb[:], bcp[:, 0:SH])
                for d in range(ND):
                    nc.vector.tensor_tensor(xn_t[d][:], x_t[d][:, cs],
                                            bc_sb[:], AB.mult)
                # fc + leaky_relu2
                for hh in range(ND):
                    hp = psB.tile([128, SH], F32, tag="psB")
                    wt = wfp.tile([128, ND, 128], F32R, tag="wfp")
                    nc.sync.dma_start(wt[:], wfc[l, hh])
                    for d in range(ND):
                        nc.tensor.matmul(hp[:, 0:SH], wt[:, d, :], xn_t[d][:],
                                         start=(d == 0), stop=(d == ND - 1))
                    hraw = gp.tile([128, SH], F32, tag="gp")
                    nc.scalar.copy(hraw[:], hp[:, 0:SH])
                    hm = gp.tile([128, SH], F32, tag="gp")
                    nc.vector.tensor_scalar(hm[:], hraw[:], 0.0, 0.01,
                                            AB.min, AB.mult)
                    h2 = gp.tile([128, SH], F32, tag="gp")
                    nc.vector.scalar_tensor_tensor(h2[:], hraw[:], 0.0,
                                                   hraw[:], AB.max, AB.mult)
                    nc.vector.scalar_tensor_tensor(h_t[hh][:, cs], h2[:], 1.0,
                                                   hm[:], AB.mult, AB.add)
                # down proj -> bf16 bounce
                for d in range(ND):
                    mlpp = psB.tile([128, SH], F32, tag="psB")
                    wt = wpp.tile([128, ND, 128], BF16, tag="wpp")
                    nc.sync.dma_start(wt[:], wp[l, d])
                    for hh in range(ND):
                        nc.tensor.matmul(mlpp[:, 0:SH], wt[:, hh, :],
                                         h_t[hh][:, cs],
                                         start=(hh == 0), stop=(hh == ND - 1))
                    mt = gpb.tile([128, SH], BF16, tag="gpb")
                    nc.scalar.copy(mt[:], mlpp[:, 0:SH])
                    nc.gpsimd.dma_start(arb_p[1][h][128 * d:128 * d + 128, :],
                                        mt[:])

            def trigger(c, h):
                nc.gpsimd.collective_compute(
                    "AllReduce", AB.add, replica_groups=RG,
                    ins=[arb_p[c][h][:, :].opt()],
                    outs=[arb_o[c][h][:, :].opt()])

            # ================= layer loop =================
            for l in range(L):
                resid_a(l)
                residmix_b(l)
                pss_b = qkv_b_partial(l)
                lif_chain(l, 0, 0)
                consume_b(l, pss_b)
                lnbc_half(l, 1)
                lif_chain(l, 0, 1)
                rope_qk(l, 1)
                lif_chain(l, 1, 0)
                attn_half(l, 0)
                lif_chain(l, 1, 1)
                wo_half(l, 0)
                trigger(0, 0)
                attn_half(l, 1)
                wo_half(l, 1)
                trigger(0, 1)
                mlp_half(l, 0)
                trigger(1, 0)
                mlp_half(l, 1)
                trigger(1, 1)
                if l < L - 1:
                    nc.sync.dma_start(wqkv_s[:], wqkv[l + 1])

            # ---- final: AR2 consume + norm + logits per half ----
            for h in range(2):
                cs = cs_of(h)
                for d in range(ND):
                    ds = slice(128 * d, 128 * d + 128)
                    mt = wop.tile([128, SH], BF16, tag="wop")
                    nc.scalar.dma_start(mt[:], arb_o[1][h][ds, :])
                    msc = scal_s[:, d, 5 * (L - 1) + 3:5 * (L - 1) + 4]
                    nc.vector.scalar_tensor_tensor(
                        x_t[d][:, cs], mt[:], msc, x_t[d][:, cs],
                        AB.mult, AB.add)
                ssq_ps = psB.tile([128, SH], F32, tag="psB")
                ssq_half(ssq_ps, h, x_t, split=True)
                nc.scalar.activation(rl_row[:, cs], ssq_ps[0:1, 0:SH],
                                     AFT.Sqrt,
                                     bias=epsc[0:1, :], scale=1.0 / DM)
                nc.vector.reciprocal_approx_fast(out=rl_row[:, cs],
                                                 in_=rl_row[:, cs])
                nc.scalar.copy(brow[0:1, 0:SH], rl_row[:, cs])
                bcp = psB.tile([128, SH], F32, tag="psB")
                nc.tensor.matmul(bcp[:, 0:SH], onesb[0:1, :], brow[0:1, 0:SH],
                                 start=True, stop=True)
                nc.scalar.copy(bc_sb[:], bcp[:, 0:SH])
                # cast normed x to bf16, reusing dead h_t space
                xnb = [h_t[d][:, 0:SH] for d in range(ND)]
                for d in range(ND):
                    nc.vector.tensor_tensor(xn_t[d][:], x_t[d][:, cs],
                                            bc_sb[:], AB.mult)
                    nc.gpsimd.tensor_copy(xnb[d], xn_t[d][:])
                for o in range(HASH_PC // 512):
                    lg_ps = [psB.tile([128, 512], F32, tag="psB",
                                      name=f"lgp{i}") for i in range(4)]
                    for d in range(ND):
                        ut = unp.tile([128, 512], BF16, tag="unp")
                        qeng = (nc.gpsimd, nc.sync, nc.scalar, nc.sync)[d % 4]
                        qeng.dma_start(ut[:],
                                       unit[128 * d:128 * d + 128,
                                            512 * o:512 * o + 512])
                        for tl in range(4):
                            nc.tensor.matmul(
                                lg_ps[tl][:, 0:512],
                                xnb[d][:, 128 * tl:128 * tl + 128],
                                ut[:], start=(d == 0), stop=(d == ND - 1))
                    for tl in range(4):
                        t = 4 * h + tl
                        ot = gpb.tile([128, 512], BF16, tag="gpb")
                        nc.scalar.copy(ot[:], lg_ps[tl][:, 0:512])
                        (nc.sync if tl % 2 else nc.gpsimd).dma_start(
                            out_lg[128 * t:128 * t + 128,
                                   512 * o:512 * o + 512], ot[:])

    nc.compile()
    return nc


def _host_prep(inputs):
    ids = np.asarray(inputs["input_ids"])
    uni = np.ascontiguousarray(inputs["uni"], np.float32)
    bi = np.ascontiguousarray(inputs["bi"], np.float32)
    Wq = np.asarray(inputs["Wq"], dtype=np.float32)
    Wk = np.asarray(inputs["Wk"], dtype=np.float32)
    Wv = np.asarray(inputs["Wv"], dtype=np.float32)
    Wo = np.asarray(inputs["Wo"], dtype=np.float32)
    Wfc = np.asarray(inputs["Wfc"], dtype=np.float32)
    Wp = np.asarray(inputs["Wp"], dtype=np.float32)
    qg = np.asarray(inputs["q_gain"], dtype=np.float32)
    asc = np.asarray(inputs["attn_scale"], dtype=np.float32)
    msc = np.asarray(inputs["mlp_scale"], dtype=np.float32)
    rmx = np.asarray(inputs["resid_mix"], dtype=np.float32)

    prev = np.concatenate([np.zeros_like(ids[:, :1]), ids[:, :-1]], axis=1)
    h1 = (ids % HASH).astype(np.int64)
    h2 = ((prev.astype(np.int64) * 31 + ids) % HASH).astype(np.int64)

    inv_freq = 1.0 / (ROPE_BASE ** (np.arange(0, DH, 2, dtype=np.float32) / DH))
    freqs = np.arange(S, dtype=np.float32)[:, None] * inv_freq[None, :]
    cos = np.cos(freqs).astype(np.float32)   # [S, 32]
    sin = np.sin(freqs).astype(np.float32)
    cos64 = np.ascontiguousarray(np.concatenate([cos, cos], axis=1).T)  # [64,S]
    sin64 = np.ascontiguousarray(np.concatenate([sin, -sin], axis=1).T)
    cosq = np.ascontiguousarray(np.tile(cos64, (2, 1)))   # [128, S]
    sinq = np.ascontiguousarray(np.tile(sin64, (2, 1)))

    # swap permutations: P~[k, m] = 1 iff k = partner(m) (partner: +-32 in 64)
    pswp = np.zeros((128, 2, 128), np.float32)
    for m in range(128):
        base = (m // 64) * 64
        partner = base + (m % 64 + 32) % 64
        pswp[partner, 0, m] = 1.0
    for m in range(64):
        pswp[(m + 32) % 64, 1, m] = 1.0
    ident = np.eye(128, dtype=np.float32)
    ident[64:128, 0:64] += np.eye(64, dtype=np.float32)
    trim = np.tril(np.ones((128, 128), np.float32)).T.copy()
    sel2f = np.zeros((2, 128), np.float32)
    sel2f[0, 0:64] = 1.0
    sel2f[1, 64:128] = 1.0
    selT = np.ascontiguousarray(sel2f.T)

    # scal columns: rm0, rm1, attn_scale, mlp_scale, rm0*msc_prev
    scal = np.zeros((128, ND, 5 * L), np.float32)
    for l in range(L):
        fold = rmx[l, 0] * (msc[l - 1] if l > 0 else 0.0)
        for v, vec in enumerate((rmx[l, 0], rmx[l, 1], asc[l], msc[l], fold)):
            scal[:, :, 5 * l + v] = np.asarray(vec).reshape(ND, 128).T

    in_maps = []
    for core in range(N_CORES):
        g, r = core // TP, core % TP
        qsl = slice(QD * r, QD * (r + 1))
        ksl = slice(KD * r, KD * (r + 1))
        hsl = slice(HID_PC * r, HID_PC * (r + 1))
        asl = slice(HASH_PC * r, HASH_PC * (r + 1))
        wqkv = np.concatenate([
            Wq[:, qsl, :].transpose(0, 2, 1),
            Wk[:, ksl, :].transpose(0, 2, 1),
            Wv[:, ksl, :].transpose(0, 2, 1)], axis=2)  # [L, DM, 384]
        wqkv_t = np.ascontiguousarray(
            wqkv.reshape(L, ND, 128, QD + 2 * KD).transpose(0, 2, 1, 3))
        woT = Wo[:, :, qsl].transpose(0, 2, 1)          # [L, 256, DM]
        wo_t = np.ascontiguousarray(
            woT.reshape(L, 2, 128, ND, 128).transpose(0, 3, 2, 1, 4))
        wfcT = Wfc[:, hsl, :].transpose(0, 2, 1)        # [L, DM, HID_PC]
        wfc_t = np.ascontiguousarray(
            wfcT.reshape(L, ND, 128, ND, 128).transpose(0, 3, 2, 1, 4))
        wpT = Wp[:, :, hsl].transpose(0, 2, 1)          # [L, HID_PC, DM]
        wp_t = np.ascontiguousarray(
            wpT.reshape(L, ND, 128, ND, 128).transpose(0, 3, 2, 1, 4)
        ).astype(BF16_NP)
        qgain = np.zeros((128, 2, L), np.float32)
        for l in range(L):
            for j in range(2):
                for hp in range(2):
                    head = HEADS_PC * r + 2 * j + hp
                    qgain[64 * hp:64 * hp + 64, j, l] = qg[l, head]
        m = dict(
            xe1=np.ascontiguousarray(uni[h1[g]].T),
            xe2=np.ascontiguousarray(bi[h2[g]].T),
            wqkv=wqkv_t,
            wo=wo_t,
            wfc=wfc_t,
            wp=wp_t,
            unit=np.ascontiguousarray(uni[asl, :].T).astype(BF16_NP),
            cosq=cosq,
            sinq=sinq,
            cosk=cos64,
            sink=sin64,
            pswp=pswp,
            ident=ident,
            tri=trim,
            selT=selT,
            sel2f=sel2f.astype(np.float16),
            scal=scal,
            qgain=qgain,
        )
        in_maps.append(m)
    return in_maps


def kernel(**inputs):
    if "nc" not in _CACHE:
        _CACHE["nc"] = build_program()
    nc = _CACHE["nc"]
    in_maps = _host_prep(inputs)
    res = run_bass_kernel_spmd(nc, in_maps, core_ids=list(range(N_CORES)),
                               trace=os.environ.get("K_TRACE", "0") == "1")
    _CACHE["res"] = res
    out = np.zeros((B, S, HASH), np.float32)
    for core in range(N_CORES):
        g, r = core // TP, core % TP
        out[g, :, HASH_PC * r:HASH_PC * (r + 1)] = res.results[core]["out_lg"]
    return out


# revision 57
# speedup vs baseline: 1.0161x; 1.0031x over previous
"""Trainium2 Bass kernel for nn_GPT_61409442398424 (4-layer spiking GPT).

Sharding: DP-2 over batch (core groups {0-3},{4-7}) x TP-4 within group
(Wq/Wk/Wv by heads, Wfc/Wp by hidden dim, uni by HASH rows for logits).

v3 design notes (on top of v2), measured 2435us -> 1605us:
- Token-halved pipeline: every layer phase (QKV/rope/LIF/attention/Wo/
  AllReduce/MLP) runs per 512-token half so each AllReduce overlaps the
  other half's compute. ARs stay 1MB bf16 (mesh regime, ~33us each).
- LIF fixpoint (KFIX=7, adds ~7e-3 rel err; K=6 would breach the 2e-2
  gate) runs per half with carry chaining via tensor_tensor_scan's AP
  `initial`. Scans/STT are DVE-only (GpSimd lacks both; its
  tensor_scalar ucode is ~7us per [128,512] op - avoid).
- Half-b QKV is precomputed against pre-AllReduce x during the fix_a
  window, then a delta pass folds in rm0*msc*AR2b after it lands, so the
  post-AR critical path only carries 24 delta matmuls + rope.
- Attention software-pipelined: exp'd score tiles produced LOOK=3 tiles
  ahead of the PV matmuls, heads interleaved; the causal tri mask runs
  on GpSimd so attention never queues behind the other half's LIF.
- Weights host-packed into exact tile layouts (>=1KB contiguous per
  partition per descriptor); wp/unit/h/logits in bf16; the q-rms,
  attention-denominator, and norm-row partition-broadcast matmuls are
  single fp16/bf16 [2,SH]-row matmuls instead of pairs of 4cyc/row f32
  (fp16 for q-rms: bf16's 8-bit mantissa wobbles softmax temperature by
  ~1.7e-2 end-to-end; fp16 is ~10x tighter).
- AR-consume tiles use a separate pool from outbound evac tiles and
  their DMAs issue from the Scalar queue; outbound bf16 evacs DMA from
  the GpSimd queue (a dma_start costs ~0.6us of its queue).
- PE runs at K=4/8 (1.2GHz) for ~60% of the span due to the HAM activity
  throttle + board GPIO caps; dense 12us+ matmul runs still measure
  cold, so further scheduling densification has limited return.
"""
import os
import numpy as np

import concourse.bass as bass
import concourse.tile as tile
from concourse import bacc, mybir
from concourse.bass_utils import run_bass_kernel_spmd

F32 = mybir.dt.float32
F32R = mybir.dt.float32r
BF16 = mybir.dt.bfloat16
FP16 = mybir.dt.float16
AB = mybir.AluOpType
AFT = mybir.ActivationFunctionType
BF16_NP = mybir.dt.np(mybir.dt.bfloat16)

B, S, DM, H, HKV, L, MLP_MULT = 2, 1024, 1024, 16, 4, 4, 4
DH = DM // H
HASH, VOCAB = 16384, 50257
EPS = 1.1920929e-07
THRESH, DECAY = 0.8, 0.9
ROPE_BASE = 10000.0
N_CORES = 8
TP = 4
HEADS_PC = H // TP        # 4 q heads per core
QD = HEADS_PC * DH        # 256 q dims per core
KD = DH                   # 64 kv dims per core (1 kv head)
HID_PC = MLP_MULT * DM // TP
HASH_PC = HASH // TP
NT = S // 128
ND = DM // 128
SH = S // 2               # tokens per half
KFIX = 7                  # LIF fixpoint scans per half

_CACHE = {}


def build_program():
    nc = bacc.Bacc("TRN2", target_bir_lowering=False, debug=False,
                   enable_asserts=False, num_devices=N_CORES)

    din = {}
    def di(name, shape, dt=F32R):
        din[name] = nc.dram_tensor(name, shape, dt, kind="ExternalInput").ap()
        return din[name]

    xe1 = di("xe1", [DM, S], F32R)
    xe2 = di("xe2", [DM, S], F32R)
    wqkv = di("wqkv", [L, 128, ND, QD + 2 * KD], F32R)  # tile-packed
    wo = di("wo", [L, ND, 128, 2, 128], F32R)           # [l,d,p,c,f]
    wfc = di("wfc", [L, ND, 128, ND, 128], F32R)        # [l,hh,p,d,f]
    wp = di("wp", [L, ND, 128, ND, 128], BF16)          # [l,d,p,hh,f]
    unit = di("unit", [DM, HASH_PC], BF16)
    cosq = di("cosq", [128, S], F32)     # q-tile rope tables (2 heads/tile)
    sinq = di("sinq", [128, S], F32)     # signed
    cosk = di("cosk", [64, S], F32)
    sink = di("sink", [64, S], F32)
    pswp = di("pswp", [128, 2, 128], F32R)  # [:,0,:]=Pq ; [0:64,1,0:64]=Pk
    ident = di("ident", [128, 128], F32R)
    tri = di("tri", [128, 128], F32R)
    selT = di("selT", [128, 2], F32R)
    sel2f = di("sel2f", [2, 128], FP16)
    scal = di("scal", [128, ND, 5 * L], F32)
    qgain = di("qgain", [128, 2, L], F32)
    out_lg = nc.dram_tensor("out_lg", [S, HASH_PC], BF16, kind="ExternalOutput").ap()

    # ---------------- persistent SBUF ------------------------------------
    x_t = [nc.alloc_sbuf_tensor(f"x_{d}", [128, S], F32R) for d in range(ND)]
    xn_t = [nc.alloc_sbuf_tensor(f"xn_{d}", [128, SH], F32R) for d in range(ND)]
    h_t = [nc.alloc_sbuf_tensor(f"h_{d}", [128, S], BF16) for d in range(ND)]
    qsb = [nc.alloc_sbuf_tensor(f"qsb_{j}", [128, S], F32R) for j in range(2)]
    kvsb = nc.alloc_sbuf_tensor("kvsb", [128, SH], F32R)
    q4 = [nc.alloc_sbuf_tensor(f"q4_{j}", [128, S], F32R) for j in range(2)]
    u2 = [nc.alloc_sbuf_tensor(f"u2_{j}", [128, S], F32) for j in range(2)]
    c2 = [nc.alloc_sbuf_tensor(f"c2_{j}", [128, S + 1], F32) for j in range(2)]
    e2 = [nc.alloc_sbuf_tensor(f"e2_{j}", [128, SH], F32) for j in range(2)]
    et2 = [nc.alloc_sbuf_tensor(f"et2_{j}", [128, SH], F32) for j in range(2)]
    yt2 = [nc.alloc_sbuf_tensor(f"yt2_{j}", [128, SH], F32R) for j in range(2)]
    v65 = nc.alloc_sbuf_tensor("v65", [128, NT, 65], F32R)
    kt2 = nc.alloc_sbuf_tensor("kt2", [128, S], F32R)
    bc_sb = nc.alloc_sbuf_tensor("bc_sb", [128, SH], F32)
    wqkv_s = nc.alloc_sbuf_tensor("wqkv_s", [128, ND, QD + 2 * KD], F32R)
    cosq_s = nc.alloc_sbuf_tensor("cosq_s", [128, S], F32)
    sinq_s = nc.alloc_sbuf_tensor("sinq_s", [128, S], F32)
    cosk_s = nc.alloc_sbuf_tensor("cosk_s", [64, S], F32)
    sink_s = nc.alloc_sbuf_tensor("sink_s", [64, S], F32)
    pswp_s = nc.alloc_sbuf_tensor("pswp_s", [128, 2, 128], F32R)
    ident_s = nc.alloc_sbuf_tensor("ident_s", [128, 128], F32R)
    tri_s = nc.alloc_sbuf_tensor("tri_s", [128, 128], F32R)
    scal_s = nc.alloc_sbuf_tensor("scal_s", [128, ND, 5 * L], F32)
    qgain_s = nc.alloc_sbuf_tensor("qgain_s", [128, 2, L], F32)
    rkc = nc.alloc_sbuf_tensor("rkc", [128, NT], F32)    # 0.125/rms(k) per key
    lnbc = nc.alloc_sbuf_tensor("lnbc", [128, NT], F32)  # ln(bc) per key
    ibc = nc.alloc_sbuf_tensor("ibc", [128, NT], F32R)   # 1/bc per key
    rows_sb = nc.alloc_sbuf_tensor("rows_sb", [128, S], F32)
    hrows = nc.alloc_sbuf_tensor("hrows", [2, SH], FP16)
    drows = nc.alloc_sbuf_tensor("drows", [33, SH], BF16)
    selT_s = nc.alloc_sbuf_tensor("selT_s", [128, 2], F32R)
    sel2f_s = nc.alloc_sbuf_tensor("sel2f_s", [2, 128], FP16)
    onesb = nc.alloc_sbuf_tensor("onesb", [33, 128], BF16)
    brow = nc.alloc_sbuf_tensor("brow", [1, SH], BF16)
    onesr_f = nc.alloc_sbuf_tensor("onesr_f", [128, 128], F32)
    onesr = nc.alloc_sbuf_tensor("onesr", [128, 128], F32R)
    onesc_f = nc.alloc_sbuf_tensor("onesc_f", [128, 1], F32)
    onesc = nc.alloc_sbuf_tensor("onesc", [128, 1], F32R)
    d9_s = nc.alloc_sbuf_tensor("d9_s", [128, 1], F32)
    mtmp = nc.alloc_sbuf_tensor("mtmp", [128, 1], F32)
    zc = nc.alloc_sbuf_tensor("zc", [128, 1], F32)
    epsc = nc.alloc_sbuf_tensor("epsc", [128, 1], F32)
    rl_row = rows_sb[0:1, :]         # 1/rms per token (norms)
    ln_row = rows_sb[64:65, :]       # ln per token (v bias)

    RG = [[0, 1, 2, 3], [4, 5, 6, 7]]

    with tile.TileContext(nc) as tc:
        with tc.tile_pool(name="gp", bufs=5) as gp, \
             tc.tile_pool(name="gpb", bufs=3) as gpb, \
             tc.tile_pool(name="etp", bufs=4) as etp, \
             tc.tile_pool(name="wop", bufs=3) as wop, \
             tc.tile_pool(name="wfp", bufs=3) as wfp, \
             tc.tile_pool(name="wpp", bufs=3) as wpp, \
             tc.tile_pool(name="unp", bufs=8) as unp, \
             tc.tile_pool(name="psB", bufs=6, space="PSUM") as psB, \
             tc.tile_pool(name="psY", bufs=2, space="PSUM") as psY, \
             tc.tile_pool(name="dram", bufs=1, space="DRAM") as dram:

            arb_i = dram.tile([DM, S], F32R)
            arb_p = [[dram.tile([DM, SH], BF16, name=f"arbp{c}{h}")
                      for h in range(2)] for c in range(2)]
            arb_o = [[dram.tile([DM, SH], BF16, name=f"arbo{c}{h}")
                      for h in range(2)] for c in range(2)]

            # ---- constants / tables ----
            nc.sync.dma_start(cosq_s[:], cosq[:])
            nc.sync.dma_start(sinq_s[:], sinq[:])
            nc.sync.dma_start(cosk_s[:], cosk[:])
            nc.sync.dma_start(sink_s[:], sink[:])
            nc.sync.dma_start(pswp_s[:], pswp[:])
            nc.sync.dma_start(ident_s[:], ident[:])
            nc.sync.dma_start(tri_s[:], tri[:])
            nc.sync.dma_start(selT_s[:], selT[:])
            nc.sync.dma_start(sel2f_s[:], sel2f[:])
            nc.sync.dma_start(scal_s[:], scal[:])
            nc.sync.dma_start(qgain_s[:], qgain[:])
            nc.sync.dma_start(wqkv_s[:], wqkv[0])
            nc.vector.memset(mtmp[:], 1.0)
            nc.vector.tensor_copy(onesc[:], mtmp[:])
            nc.vector.tensor_copy(onesr[:], mtmp[:, 0:1].to_broadcast((128, 128)))
            nc.vector.tensor_copy(onesr_f[:], mtmp[:, 0:1].to_broadcast((128, 128)))
            nc.vector.tensor_copy(onesb[:], mtmp[0:33, 0:1].to_broadcast((33, 128)))
            nc.vector.tensor_copy(onesc_f[:], mtmp[:])
            nc.vector.memset(d9_s[:], 0.9)
            nc.vector.memset(zc[:], 0.0)
            nc.vector.memset(epsc[:], EPS)

            # ---- embedding: x = xe1 + xe2 (also x0, kept in DRAM) ----
            for d in range(ND):
                ds = slice(128 * d, 128 * d + 128)
                nc.sync.dma_start(x_t[d][:], xe2[ds, :])
                for eh in range(2):
                    ecs = slice(SH * eh, SH * eh + SH)
                    t1 = gp.tile([128, SH], F32R, tag="gp")
                    nc.sync.dma_start(t1[:], xe1[ds, ecs])
                    nc.gpsimd.tensor_tensor(x_t[d][:, ecs], x_t[d][:, ecs],
                                            t1[:], AB.add)
                    nc.sync.dma_start(arb_i[ds, ecs], x_t[d][:, ecs])

            def cs_of(h):
                return slice(SH * h, SH * h + SH)

            def ssq_half(ps, h, src, split=False):
                """ps[0:1, 0:SH] = sum over DM of src^2 for token half h."""
                for d in range(ND):
                    sq = gp.tile([128, SH], F32R, tag="gp")
                    if split and d % 2 == 1:
                        nc.vector.tensor_tensor(sq[:], src[d][:, cs_of(h)],
                                                src[d][:, cs_of(h)], AB.mult)
                    else:
                        nc.scalar.activation(sq[:], src[d][:, cs_of(h)],
                                             AFT.Square, bias=zc[:])
                    nc.tensor.matmul(ps[0:1, 0:SH], onesc[:], sq[:],
                                     start=(d == 0), stop=(d == ND - 1))

            def lnbc_half(l, h):
                """Per-token ln(rsqrt(mean x^2+eps)) for v (exp bias), half h."""
                cs = cs_of(h)
                ssq_ps = psB.tile([128, SH], F32, tag="psB")
                ssq_half(ssq_ps, h, x_t)
                nc.scalar.activation(ln_row[:, cs], ssq_ps[0:1, 0:SH], AFT.Ln,
                                     bias=epsc[0:1, :], scale=1.0 / DM)
                nc.vector.tensor_scalar(ln_row[:, cs], ln_row[:, cs], -0.5,
                                        None, AB.mult)
                lnp = psB.tile([128, SH], F32, tag="psB")
                for tl in range(4):
                    t = 4 * h + tl
                    nc.tensor.transpose(lnp[:, tl:tl + 1],
                                        rows_sb[64:65, 128 * t:128 * t + 128]
                                        .bitcast(F32),
                                        ident_s[64:65, 64:65].bitcast(F32))
                nc.scalar.copy(lnbc[:, 4 * h:4 * h + 4], lnp[:, 0:4])
                nc.scalar.activation(ibc[:, 4 * h:4 * h + 4],
                                     lnbc[:, 4 * h:4 * h + 4], AFT.Exp,
                                     bias=zc[:], scale=-1.0)

            def rope_qk(l, h):
                """q-head rms + rope, k rope + rms, v transpose for half h."""
                cs = cs_of(h)
                for jt in range(2):
                    sq = gp.tile([128, SH], F32R, tag="gp")
                    nc.scalar.activation(sq[:], qsb[jt][:, cs], AFT.Square,
                                         bias=zc[:])
                    rq_ps = psB.tile([128, SH], F32, tag="psB")
                    nc.tensor.matmul(rq_ps[0:2, 0:SH], selT_s[:], sq[:],
                                     start=True, stop=True)
                    nc.scalar.activation(hrows[0:2, 0:SH], rq_ps[0:2, 0:SH],
                                         AFT.Sqrt, bias=zc[0:2, :],
                                         scale=1.0 / DH)
                    rqb = psB.tile([128, SH], F32, tag="psB")
                    nc.tensor.matmul(rqb[:, 0:SH], sel2f_s[:],
                                     hrows[0:2, 0:SH], start=True, stop=True)
                    rqi = gp.tile([128, SH], F32, tag="gp", name="rqi")
                    nc.vector.reciprocal_approx_fast(out=rqi[:],
                                                     in_=rqb[:, 0:SH])
                    swp = psB.tile([128, SH], F32, tag="psB")
                    nc.tensor.matmul(swp[:, 0:SH], pswp_s[:, 0, :],
                                     qsb[jt][:, cs], start=True, stop=True)
                    t1 = et2[0][:, 0:SH]
                    nc.vector.scalar_tensor_tensor(
                        t1, qsb[jt][:, cs], 1.0, cosq_s[:, cs],
                        AB.mult, AB.mult)
                    t2 = et2[1][:, 0:SH]
                    nc.vector.scalar_tensor_tensor(
                        t2, swp[:, 0:SH], 1.0, sinq_s[:, cs],
                        AB.mult, AB.mult)
                    nc.vector.scalar_tensor_tensor(
                        t1, t1, 1.0, t2, AB.mult, AB.add)
                    nc.vector.scalar_tensor_tensor(
                        qsb[jt][:, cs], t1, 1.0, rqi[:], AB.mult, AB.mult)

                # k rope
                swp = psB.tile([128, SH], F32, tag="psB")
                nc.tensor.matmul(swp[0:64, 0:SH], pswp_s[0:64, 1, 0:64],
                                 kvsb[0:64, 0:SH], start=True, stop=True)
                t1 = et2[0][0:64, 0:SH]
                nc.vector.scalar_tensor_tensor(
                    t1, kvsb[0:64, 0:SH], 1.0, cosk_s[:, cs],
                    AB.mult, AB.mult)
                t2 = et2[1][0:64, 0:SH]
                nc.vector.scalar_tensor_tensor(
                    t2, swp[0:64, 0:SH], 1.0, sink_s[:, cs],
                    AB.mult, AB.mult)
                nc.vector.scalar_tensor_tensor(
                    kt2[0:64, cs], t1, 1.0, t2, AB.mult, AB.add)
                nc.scalar.copy(kt2[64:128, cs], kt2[0:64, cs])

                # k-head rms -> per-key scale column (0.125/rms)
                ksq = gp.tile([128, SH], F32, tag="gp")
                nc.scalar.activation(ksq[0:64, :], kt2[0:64, cs], AFT.Square,
                                     bias=zc[0:64, :])
                rkp = psB.tile([128, SH], F32, tag="psB")
                for tl in range(4):
                    nc.tensor.matmul(rkp[:, tl:tl + 1],
                                     ksq[0:64, 128 * tl:128 * tl + 128],
                                     onesc_f[0:64, :],
                                     start=True, stop=True)
                tsl = slice(4 * h, 4 * h + 4)
                nc.scalar.activation(rkc[:, tsl], rkp[:, 0:4], AFT.Sqrt,
                                     bias=zc[:], scale=1.0 / DH)
                nc.vector.reciprocal_approx_fast(out=rkc[:, tsl],
                                                 in_=rkc[:, tsl])
                nc.vector.tensor_scalar(rkc[:, tsl], rkc[:, tsl], 0.125,
                                        None, AB.mult)
                # v -> token-major tiles via PE transpose
                for tl in range(4):
                    t = 4 * h + tl
                    vtp = psB.tile([128, SH], F32, tag="psB")
                    nc.tensor.transpose(vtp[:, 0:64].bitcast(F32R),
                                        kvsb[64:128, 128 * tl:128 * tl + 128],
                                        ident_s[64:128, 0:64])
                    nc.scalar.copy(v65[:, t, 0:64], vtp[:, 0:64])
                    nc.gpsimd.tensor_copy(v65[:, t, 64:65], ibc[:, t:t + 1])

            def resid_a(l):
                """AR2a consume + resid mix + QKV + lnbc + rope for half a."""
                cs = cs_of(0)
                for d in range(ND):
                    ds = slice(128 * d, 128 * d + 128)
                    rm0 = scal_s[:, d, 5 * l + 0:5 * l + 1]
                    rm1 = scal_s[:, d, 5 * l + 1:5 * l + 2]
                    if l == 0:
                        tt = gp.tile([128, SH], F32, tag="gp")
                        nc.scalar.mul(tt[:], x_t[d][:, cs], rm1)
                        nc.vector.scalar_tensor_tensor(
                            x_t[d][:, cs], x_t[d][:, cs], rm0, tt[:],
                            AB.mult, AB.add)
                    else:
                        x0t = gp.tile([128, SH], F32R, tag="gp")
                        nc.sync.dma_start(x0t[:], arb_i[ds, cs])
                        tt = gp.tile([128, SH], F32, tag="gp")
                        nc.scalar.mul(tt[:], x0t[:], rm1)
                        nc.vector.scalar_tensor_tensor(
                            x_t[d][:, cs], x_t[d][:, cs], rm0, tt[:],
                            AB.mult, AB.add)
                        art = wop.tile([128, SH], BF16, tag="wop")
                        nc.scalar.dma_start(art[:], arb_o[1][0][ds, :])
                        fold = scal_s[:, d, 5 * l + 4:5 * l + 5]
                        nc.vector.scalar_tensor_tensor(
                            x_t[d][:, cs], art[:], fold, x_t[d][:, cs],
                            AB.mult, AB.add)
                pss = [psB.tile([128, SH], F32, tag="psB", name=f"qkva{i}")
                       for i in range(3)]
                for d in range(ND):
                    for jt in range(3):
                        nc.tensor.matmul(
                            pss[jt][:, 0:SH],
                            wqkv_s[:, d, 128 * jt:128 * jt + 128],
                            x_t[d][:, cs],
                            start=(d == 0), stop=(d == ND - 1))
                for jt in range(2):
                    nc.scalar.copy(qsb[jt][:, cs], pss[jt][:, 0:SH])
                nc.scalar.copy(kvsb[:, 0:SH], pss[2][:, 0:SH])
                lnbc_half(l, 0)
                rope_qk(l, 0)

            def residmix_b(l):
                """x_b = rm0*x_b + rm1*x0_b (AR2b part folded in later)."""
                cs = cs_of(1)
                for d in range(ND):
                    ds = slice(128 * d, 128 * d + 128)
                    rm0 = scal_s[:, d, 5 * l + 0:5 * l + 1]
                    rm1 = scal_s[:, d, 5 * l + 1:5 * l + 2]
                    if l == 0:
                        tt = gp.tile([128, SH], F32, tag="gp")
                        nc.scalar.mul(tt[:], x_t[d][:, cs], rm1)
                        nc.vector.scalar_tensor_tensor(
                            x_t[d][:, cs], x_t[d][:, cs], rm0, tt[:],
                            AB.mult, AB.add)
                    else:
                        x0t = gp.tile([128, SH], F32R, tag="gp")
                        nc.sync.dma_start(x0t[:], arb_i[ds, cs])
                        tt = gp.tile([128, SH], F32, tag="gp")
                        nc.scalar.mul(tt[:], x0t[:], rm1)
                        nc.vector.scalar_tensor_tensor(
                            x_t[d][:, cs], x_t[d][:, cs], rm0, tt[:],
                            AB.mult, AB.add)

            def qkv_b_partial(l):
                """QKV over pre-AR x_b; for l=0 this is the whole thing."""
                cs = cs_of(1)
                pss = [psB.tile([128, SH], F32, tag="psB", name=f"qkvb{i}")
                       for i in range(3)]
                for d in range(ND):
                    for jt in range(3):
                        nc.tensor.matmul(
                            pss[jt][:, 0:SH],
                            wqkv_s[:, d, 128 * jt:128 * jt + 128],
                            x_t[d][:, cs],
                            start=(d == 0),
                            stop=(l == 0 and d == ND - 1))
                if l == 0:
                    for jt in range(2):
                        nc.scalar.copy(qsb[jt][:, cs], pss[jt][:, 0:SH])
                    nc.scalar.copy(kvsb[:, 0:SH], pss[2][:, 0:SH])
                return pss

            def consume_b(l, pss):
                """Fold AR2b into x_b and into the QKV_b psums (delta pass)."""
                if l == 0:
                    return
                cs = cs_of(1)
                for d in range(ND):
                    ds = slice(128 * d, 128 * d + 128)
                    art = wop.tile([128, SH], BF16, tag="wop")
                    nc.scalar.dma_start(art[:], arb_o[1][1][ds, :])
                    fold = scal_s[:, d, 5 * l + 4:5 * l + 5]
                    tmp = gp.tile([128, SH], F32R, tag="gp", name="artmp")
                    nc.scalar.mul(tmp[:], art[:], fold)
                    nc.vector.tensor_tensor(x_t[d][:, cs], x_t[d][:, cs],
                                            tmp[:], AB.add)
                    for jt in range(3):
                        nc.tensor.matmul(
                            pss[jt][:, 0:SH],
                            wqkv_s[:, d, 128 * jt:128 * jt + 128],
                            tmp[:],
                            start=False, stop=(d == ND - 1))
                for jt in range(2):
                    nc.scalar.copy(qsb[jt][:, cs], pss[jt][:, 0:SH])
                nc.scalar.copy(kvsb[:, 0:SH], pss[2][:, 0:SH])

            def lif_chain(l, h, j):
                """LIF fixpoint for half h, chain j (DVE)."""
                cs = cs_of(h)
                h0, h1 = SH * h, SH * h + SH
                d9 = d9_s[:].to_broadcast((128, SH))
                z9 = zc[:].to_broadcast((128, SH))
                if True:
                    eng = nc.vector
                    if h == 0:
                        eng.tensor_tensor_scan(u2[j][:, cs], d9, qsb[j][:, cs],
                                               0.0, AB.mult, AB.add)
                        eng.memset(c2[j][:, 0:1], 0.0)
                    else:
                        eng.tensor_tensor_scan(u2[j][:, cs], d9, qsb[j][:, cs],
                                               u2[j][:, h0 - 1:h0],
                                               AB.mult, AB.add)
                        # decay-only carry fill of c cols before iteration 0
                        eng.tensor_tensor_scan(c2[j][:, h0 + 1:h1 + 1], d9, z9,
                                               c2[j][:, h0:h0 + 1],
                                               AB.mult, AB.max)
                    for p in range(KFIX):
                        if p == 0 and h == 0:
                            eng.scalar_tensor_tensor(
                                e2[j][:], u2[j][:, cs], THRESH, u2[j][:, cs],
                                AB.is_ge, AB.mult)
                        else:
                            eng.scalar_tensor_tensor(
                                e2[j][:], c2[j][:, h0:h1], -DECAY,
                                u2[j][:, cs], AB.mult, AB.add)
                            eng.scalar_tensor_tensor(
                                e2[j][:], e2[j][:], THRESH, u2[j][:, cs],
                                AB.is_ge, AB.mult)
                        init = 0.0 if h == 0 else c2[j][:, h0:h0 + 1]
                        eng.tensor_tensor_scan(
                            c2[j][:, h0 + 1:h1 + 1], d9, e2[j][:], init,
                            AB.mult, AB.max)
                    # final spikes*gain; gated q -> q4
                    eng.scalar_tensor_tensor(
                        e2[j][:], c2[j][:, h0:h1], -DECAY, u2[j][:, cs],
                        AB.mult, AB.add)
                    eng.tensor_scalar(e2[j][:], e2[j][:], THRESH,
                                      qgain_s[:, j, l:l + 1],
                                      AB.is_ge, AB.mult)
                    eng.scalar_tensor_tensor(q4[j][:, cs], qsb[j][:, cs], 1.0,
                                             e2[j][:], AB.mult, AB.mult)

            def attn_half(l, h):
                """Attention for token half h (keys 0..(h+1)*SH), both chains.

                Software-pipelined: exp'd score tiles (et) are produced LOOK
                tiles ahead of the PV matmuls that consume them, with the two
                heads interleaved, so the PE never stalls on Exp latency.
                The causal tri mask runs on GpSimd (DVE is busy with the
                other half's LIF fixpoint)."""
                cs = cs_of(h)
                tlist = list(range(4 * (h + 1)))
                nmm = len(tlist)
                for j in range(2):
                    yups = [psY.tile([128, SH], F32, tag="psY", name=f"yup{i}")
                            for i in range(2)]
                    work = [(hl, t) for t in tlist for hl in range(2)]
                    ets = {}

                    def issue_score(idx):
                        hl, t = work[idx]
                        off = 64 * hl
                        tok0 = max(SH * h, 128 * t)
                        ncols = SH * h + SH - tok0
                        et = etp.tile([128, SH], F32R, tag="etp")
                        scp = psB.tile([128, SH], F32, tag="psB")
                        nc.tensor.matmul(
                            scp[:, 0:ncols],
                            kt2[off:off + 64, 128 * t:128 * t + 128],
                            q4[j][off:off + 64, tok0:SH * h + SH],
                            start=True, stop=True)
                        nc.scalar.activation(
                            et[:, 0:ncols], scp[:, 0:ncols],
                            AFT.Exp, bias=lnbc[:, t:t + 1],
                            scale=rkc[:, t:t + 1])
                        if 128 * t >= SH * h:
                            nc.gpsimd.tensor_tensor(
                                et[:, 0:128], et[:, 0:128], tri_s[:],
                                AB.mult)
                        ets[idx] = et

                    LOOK = 3
                    for idx in range(min(LOOK, len(work))):
                        issue_score(idx)
                    for idx, (hl, t) in enumerate(work):
                        if idx + LOOK < len(work):
                            issue_score(idx + LOOK)
                        et = ets.pop(idx)
                        tok0 = max(SH * h, 128 * t)
                        ncols = SH * h + SH - tok0
                        cols0 = tok0 - SH * h
                        i = tlist.index(t)
                        nc.tensor.matmul(yups[hl][0:65, cols0:SH],
                                         v65[:, t, :], et[:, 0:ncols],
                                         start=(i == 0), stop=(i == nmm - 1))
                    for hl in range(2):
                        off = 64 * hl
                        yup = yups[hl]
                        nc.scalar.copy(q4[j][off:off + 64, cs], yup[0:64, :])
                        nc.scalar.copy(drows[32 * hl:32 * hl + 1, 0:SH],
                                       yup[64:65, :])
                    # epilogue: divide by denominator broadcast
                    rbp = psB.tile([128, SH], F32, tag="psB")
                    for hl in range(2):
                        nc.tensor.matmul(rbp[64 * hl:64 * hl + 64, 0:SH],
                                         onesb[32 * hl:32 * hl + 1, 0:64],
                                         drows[32 * hl:32 * hl + 1, 0:SH],
                                         start=True, stop=True)
                    rbi = gp.tile([128, SH], F32, tag="gp", name="rbi")
                    nc.vector.reciprocal_approx_fast(out=rbi[:], in_=rbp[:, 0:SH])
                    nc.vector.scalar_tensor_tensor(yt2[j][:, 0:SH],
                                                   q4[j][:, cs], 1.0,
                                                   rbi[:], AB.mult, AB.mult)

            def wo_half(l, h):
                """Wo partials for half h -> bf16 bounce; caller triggers AR."""
                cs = cs_of(h)
                for d in range(ND):
                    aop = psB.tile([128, SH], F32, tag="psB")
                    wt = wop.tile([128, 2, 128], F32R, tag="wop")
                    nc.sync.dma_start(wt[:], wo[l, d])
                    for c in range(2):
                        nc.tensor.matmul(aop[:, 0:SH], wt[:, c, :],
                                         yt2[c][:, 0:SH],
                                         start=(c == 0), stop=(c == 1))
                    att = gpb.tile([128, SH], BF16, tag="gpb")
                    nc.scalar.copy(att[:], aop[:, 0:SH])
                    nc.gpsimd.dma_start(arb_p[0][h][128 * d:128 * d + 128, :],
                                        att[:])

            def mlp_half(l, h):
                """AR1 consume + MLP for half h -> bf16 bounce for AR2."""
                cs = cs_of(h)
                for d in range(ND):
                    ds = slice(128 * d, 128 * d + 128)
                    att = wop.tile([128, SH], BF16, tag="wop")
                    nc.scalar.dma_start(att[:], arb_o[0][h][ds, :])
                    asc = scal_s[:, d, 5 * l + 2:5 * l + 3]
                    nc.vector.scalar_tensor_tensor(
                        x_t[d][:, cs], att[:], asc, x_t[d][:, cs],
                        AB.mult, AB.add)
                # rmsnorm -> xn (half-width buffers)
                ssq_ps = psB.tile([128, SH], F32, tag="psB")
                ssq_half(ssq_ps, h, x_t, split=True)
                nc.scalar.activation(rl_row[:, cs], ssq_ps[0:1, 0:SH],
                                     AFT.Sqrt,
                                     bias=epsc[0:1, :], scale=1.0 / DM)
                nc.vector.reciprocal_approx_fast(out=rl_row[:, cs],
                                                 in_=rl_row[:, cs])
                nc.scalar.copy(brow[0:1, 0:SH], rl_row[:, cs])
                bcp = psB.tile([128, SH], F32, tag="psB")
                nc.tensor.matmul(bcp[:, 0:SH], onesb[0:1, :], brow[0:1, 0:SH],
                                 start=True, stop=True)
                nc.scalar.copy(bc_sb[:], bcp[:, 0:SH])
                for d in range(ND):
                    nc.vector.tensor_tensor(xn_t[d][:], x_t[d][:, cs],
                                            bc_sb[:], AB.mult)
                # fc + leaky_relu2
                for hh in range(ND):
                    hp = psB.tile([128, SH], F32, tag="psB")
                    wt = wfp.tile([128, ND, 128], F32R, tag="wfp")
                    nc.sync.dma_start(wt[:], wfc[l, hh])
                    for d in range(ND):
                        nc.tensor.matmul(hp[:, 0:SH], wt[:, d, :], xn_t[d][:],
                                         start=(d == 0), stop=(d == ND - 1))
                    hraw = gp.tile([128, SH], F32, tag="gp")
                    nc.scalar.copy(hraw[:], hp[:, 0:SH])
                    hm = gp.tile([128, SH], F32, tag="gp")
                    nc.vector.tensor_scalar(hm[:], hraw[:], 0.0, 0.01,
                                            AB.min, AB.mult)
                    h2 = gp.tile([128, SH], F32, tag="gp")
                    nc.vector.scalar_tensor_tensor(h2[:], hraw[:], 0.0,
                                                   hraw[:], AB.max, AB.mult)
                    nc.vector.scalar_tensor_tensor(h_t[hh][:, cs], h2[:], 1.0,
                                                   hm[:], AB.mult, AB.add)
                # down proj -> bf16 bounce
                for d in range(ND):
                    mlpp = psB.tile([128, SH], F32, tag="psB")
                    wt = wpp.tile([128, ND, 128], BF16, tag="wpp")
                    nc.sync.dma_start(wt[:], wp[l, d])
                    for hh in range(ND):
                        nc.tensor.matmul(mlpp[:, 0:SH], wt[:, hh, :],
                                         h_t[hh][:, cs],
                                         start=(hh == 0), stop=(hh == ND - 1))
                    mt = gpb.tile([128, SH], BF16, tag="gpb")
                    nc.scalar.copy(mt[:], mlpp[:, 0:SH])
                    nc.gpsimd.dma_start(arb_p[1][h][128 * d:128 * d + 128, :],
                                        mt[:])

            def trigger(c, h):
                nc.gpsimd.collective_compute(
                    "AllReduce", AB.add, replica_groups=RG,
                    ins=[arb_p[c][h][:, :].opt()],
                    outs=[arb_o[c][h][:, :].opt()])

            # ================= layer loop =================
            for l in range(L):
                resid_a(l)
                residmix_b(l)
                pss_b = qkv_b_partial(l)
                lif_chain(l, 0, 0)
                consume_b(l, pss_b)
                lnbc_half(l, 1)
                lif_chain(l, 0, 1)
                rope_qk(l, 1)
                lif_chain(l, 1, 0)
                attn_half(l, 0)
                lif_chain(l, 1, 1)
                wo_half(l, 0)
                trigger(0, 0)
                attn_half(l, 1)
                wo_half(l, 1)
                trigger(0, 1)
                mlp_half(l, 0)
                trigger(1, 0)
                mlp_half(l, 1)
                trigger(1, 1)
                if l < L - 1:
                    nc.sync.dma_start(wqkv_s[:], wqkv[l + 1])

            # ---- final: AR2 consume + norm + logits per half ----
            for h in range(2):
                cs = cs_of(h)
                for d in range(ND):
                    ds = slice(128 * d, 128 * d + 128)
                    mt = wop.tile([128, SH], BF16, tag="wop")
                    nc.scalar.dma_start(mt[:], arb_o[1][h][ds, :])
                    msc = scal_s[:, d, 5 * (L - 1) + 3:5 * (L - 1) + 4]
                    nc.vector.scalar_tensor_tensor(
                        x_t[d][:, cs], mt[:], msc, x_t[d][:, cs],
                        AB.mult, AB.add)
                ssq_ps = psB.tile([128, SH], F32, tag="psB")
                ssq_half(ssq_ps, h, x_t, split=True)
                nc.scalar.activation(rl_row[:, cs], ssq_ps[0:1, 0:SH],
                                     AFT.Sqrt,
                                     bias=epsc[0:1, :], scale=1.0 / DM)
                nc.vector.reciprocal_approx_fast(out=rl_row[:, cs],
                                                 in_=rl_row[:, cs])
                nc.scalar.copy(brow[0:1, 0:SH], rl_row[:, cs])
                bcp = psB.tile([128, SH], F32, tag="psB")
                nc.tensor.matmul(bcp[:, 0:SH], onesb[0:1, :], brow[0:1, 0:SH],
                                 start=True, stop=True)
                nc.scalar.copy(bc_sb[:], bcp[:, 0:SH])
                # cast normed x to bf16, reusing dead h_t space
                xnb = [h_t[d][:, 0:SH] for d in range(ND)]
                for d in range(ND):
                    nc.vector.tensor_tensor(xn_t[d][:], x_t[d][:, cs],
                                            bc_sb[:], AB.mult)
                    nc.gpsimd.tensor_copy(xnb[d], xn_t[d][:])
                for o in range(HASH_PC // 512):
                    lg_ps = [psB.tile([128, 512], F32, tag="psB",
                                      name=f"lgp{i}") for i in range(4)]
                    for d in range(ND):
                        ut = unp.tile([128, 512], BF16, tag="unp")
                        qeng = (nc.gpsimd, nc.sync, nc.scalar, nc.sync)[d % 4]
                        qeng.dma_start(ut[:],
                                       unit[128 * d:128 * d + 128,
                                            512 * o:512 * o + 512])
                        for tl in range(4):
                            nc.tensor.matmul(
                                lg_ps[tl][:, 0:512],
                                xnb[d][:, 128 * tl:128 * tl + 128],
                                ut[:], start=(d == 0), stop=(d == ND - 1))
                    for tl in range(4):
                        t = 4 * h + tl
                        ot = gpb.tile([128, 512], BF16, tag="gpb")
                        nc.scalar.copy(ot[:], lg_ps[tl][:, 0:512])
                        (nc.sync if tl % 2 else nc.gpsimd).dma_start(
                            out_lg[128 * t:128 * t + 128,
                                   512 * o:512 * o + 512], ot[:])

    nc.compile()
    return nc


def _host_prep(inputs):
    ids = np.asarray(inputs["input_ids"])
    uni = np.ascontiguousarray(inputs["uni"], np.float32)
    bi = np.ascontiguousarray(inputs["bi"], np.float32)
    Wq = np.asarray(inputs["Wq"], dtype=np.float32)
    Wk = np.asarray(inputs["Wk"], dtype=np.float32)
    Wv = np.asarray(inputs["Wv"], dtype=np.float32)
    Wo = np.asarray(inputs["Wo"], dtype=np.float32)
    Wfc = np.asarray(inputs["Wfc"], dtype=np.float32)
    Wp = np.asarray(inputs["Wp"], dtype=np.float32)
    qg = np.asarray(inputs["q_gain"], dtype=np.float32)
    asc = np.asarray(inputs["attn_scale"], dtype=np.float32)
    msc = np.asarray(inputs["mlp_scale"], dtype=np.float32)
    rmx = np.asarray(inputs["resid_mix"], dtype=np.float32)

    prev = np.concatenate([np.zeros_like(ids[:, :1]), ids[:, :-1]], axis=1)
    h1 = (ids % HASH).astype(np.int64)
    h2 = ((prev.astype(np.int64) * 31 + ids) % HASH).astype(np.int64)

    inv_freq = 1.0 / (ROPE_BASE ** (np.arange(0, DH, 2, dtype=np.float32) / DH))
    freqs = np.arange(S, dtype=np.float32)[:, None] * inv_freq[None, :]
    cos = np.cos(freqs).astype(np.float32)   # [S, 32]
    sin = np.sin(freqs).astype(np.float32)
    cos64 = np.ascontiguousarray(np.concatenate([cos, cos], axis=1).T)  # [64,S]
    sin64 = np.ascontiguousarray(np.concatenate([sin, -sin], axis=1).T)
    cosq = np.ascontiguousarray(np.tile(cos64, (2, 1)))   # [128, S]
    sinq = np.ascontiguousarray(np.tile(sin64, (2, 1)))

    # swap permutations: P~[k, m] = 1 iff k = partner(m) (partner: +-32 in 64)
    pswp = np.zeros((128, 2, 128), np.float32)
    for m in range(128):
        base = (m // 64) * 64
        partner = base + (m % 64 + 32) % 64
        pswp[partner, 0, m] = 1.0
    for m in range(64):
        pswp[(m + 32) % 64, 1, m] = 1.0
    ident = np.eye(128, dtype=np.float32)
    ident[64:128, 0:64] += np.eye(64, dtype=np.float32)
    trim = np.tril(np.ones((128, 128), np.float32)).T.copy()
    sel2f = np.zeros((2, 128), np.float32)
    sel2f[0, 0:64] = 1.0
    sel2f[1, 64:128] = 1.0
    selT = np.ascontiguousarray(sel2f.T)

    # scal columns: rm0, rm1, attn_scale, mlp_scale, rm0*msc_prev
    scal = np.zeros((128, ND, 5 * L), np.float32)
    for l in range(L):
        fold = rmx[l, 0] * (msc[l - 1] if l > 0 else 0.0)
        for v, vec in enumerate((rmx[l, 0], rmx[l, 1], asc[l], msc[l], fold)):
            scal[:, :, 5 * l + v] = np.asarray(vec).reshape(ND, 128).T

    in_maps = []
    for core in range(N_CORES):
        g, r = core // TP, core % TP
        qsl = slice(QD * r, QD * (r + 1))
        ksl = slice(KD * r, KD * (r + 1))
        hsl = slice(HID_PC * r, HID_PC * (r + 1))
        asl = slice(HASH_PC * r, HASH_PC * (r + 1))
        wqkv = np.concatenate([
            Wq[:, qsl, :].transpose(0, 2, 1),
            Wk[:, ksl, :].transpose(0, 2, 1),
            Wv[:, ksl, :].transpose(0, 2, 1)], axis=2)  # [L, DM, 384]
        wqkv_t = np.ascontiguousarray(
            wqkv.reshape(L, ND, 128, QD + 2 * KD).transpose(0, 2, 1, 3))
        woT = Wo[:, :, qsl].transpose(0, 2, 1)          # [L, 256, DM]
        wo_t = np.ascontiguousarray(
            woT.reshape(L, 2, 128, ND, 128).transpose(0, 3, 2, 1, 4))
        wfcT = Wfc[:, hsl, :].transpose(0, 2, 1)        # [L, DM, HID_PC]
        wfc_t = np.ascontiguousarray(
            wfcT.reshape(L, ND, 128, ND, 128).transpose(0, 3, 2, 1, 4))
        wpT = Wp[:, :, hsl].transpose(0, 2, 1)          # [L, HID_PC, DM]
        wp_t = np.ascontiguousarray(
            wpT.reshape(L, ND, 128, ND, 128).transpose(0, 3, 2, 1, 4)
        ).astype(BF16_NP)
        qgain = np.zeros((128, 2, L), np.float32)
        for l in range(L):
            for j in range(2):
                for hp in range(2):
                    head = HEADS_PC * r + 2 * j + hp
                    qgain[64 * hp:64 * hp + 64, j, l] = qg[l, head]
        m = dict(
            xe1=np.ascontiguousarray(uni[h1[g]].T),
            xe2=np.ascontiguousarray(bi[h2[g]].T),
            wqkv=wqkv_t,
            wo=wo_t,
            wfc=wfc_t,
            wp=wp_t,
            unit=np.ascontiguousarray(uni[asl, :].T).astype(BF16_NP),
            cosq=cosq,
            sinq=sinq,
            cosk=cos64,
            sink=sin64,
            pswp=pswp,
            ident=ident,
            tri=trim,
            selT=selT,
            sel2f=sel2f.astype(np.float16),
            scal=scal,
            qgain=qgain,
        )
        in_maps.append(m)
    return in_maps


def kernel(**inputs):
    if "nc" not in _CACHE:
        _CACHE["nc"] = build_program()
    nc = _CACHE["nc"]
    in_maps = _host_prep(inputs)
    res = run_bass_kernel_spmd(nc, in_maps, core_ids=list(range(N_CORES)),
                               trace=os.environ.get("K_TRACE", "0") == "1")
    _CACHE["res"] = res
    out = np.zeros((B, S, HASH), np.float32)
    for core in range(N_CORES):
        g, r = core // TP, core % TP
        out[g, :, HASH_PC * r:HASH_PC * (r + 1)] = res.results[core]["out_lg"]
    return out
